# revision 1
# baseline (speedup 1.0000x reference)
"""Block sliding-window attention on 8 TRN2 NeuronCores.

Sharding: sequence-parallel. 8 shards = (batch b in {0,1}) x (quarter s in
0..3); each core owns 2048 consecutive tokens of one batch plus a 256-token
K/V halo from the previous quarter (zeros + -inf gate for the first quarter).
No collectives: each core computes its tokens' full output rows.

Per-core pipeline (all matmuls in float32r: full PE rate, ~1e-4 rounding):
  P1: QT/KT = W^T @ hiddenT (head-transposed layout, raw), V = hidden @ Wv
      (natural layout), all staged through DRAM scratch.
  P2: per 256-token chunk: RoPE on Q/K (rot-half via partition-offset DMA
      reload + pre-signed sin), then per head: S^T = K Q^T per 128-key block,
      exp on ACT (scale=1/sqrt(128), -1e30 bias gates the no-previous case),
      0/1 triangular mask multiply on DVE (also retypes to f32r), denominator
      via all-ones matmul (broadcasts across partitions), O^T = V^T P^T,
      normalize with DVE reciprocal.
  P3: out = sum_h O_h @ Wo_h, accumulated over all 16 head blocks in PSUM.
"""
import sys

try:
    import concourse  # noqa: F401
except ImportError:
    sys.path.insert(0, '/opt/trn_rl_repo')

import ml_dtypes
import numpy as np

import concourse.bacc as bacc
import concourse.mybir as mybir
import concourse.tile as tile
from concourse.bass_utils import run_bass_kernel_spmd

f32 = mybir.dt.float32
f32r = mybir.dt.float32r
AF = mybir.ActivationFunctionType
bf16 = mybir.dt.bfloat16

DIMS = 2048
HEADS = 16
HD = 128           # head dim
WIN = 256          # window / chunk
B, S = 2, 8192
NSH = 4            # seq shards per batch
THETA = 10000.0
ISQ = float(1.0 / np.sqrt(HD))
IB = DIMS // 128   # 16 input-dim blocks


def build(nc, T, phases=(1, 2, 3)):
    """Emit the per-core program. T = local tokens (multiple of 512)."""
    TH = T + WIN                      # with halo
    NC_ = T // WIN                    # chunks
    HT = nc.dram_tensor("HT", [DIMS, TH], f32r, kind="ExternalInput")
    WQ = nc.dram_tensor("WQ", [DIMS, DIMS], f32r, kind="ExternalInput")
    WK = nc.dram_tensor("WK", [DIMS, DIMS], f32r, kind="ExternalInput")
    WV = nc.dram_tensor("WV", [DIMS, DIMS], f32r, kind="ExternalInput")
    WO = nc.dram_tensor("WO", [DIMS, DIMS], f32r, kind="ExternalInput")
    COS = nc.dram_tensor("COS", [HD, TH], f32, kind="ExternalInput")
    SINS = nc.dram_tensor("SINS", [HD, TH], f32, kind="ExternalInput")
    TRI23 = nc.dram_tensor("TRI23", [128, 2 * WIN], bf16, kind="ExternalInput")
    PGATE = nc.dram_tensor("PGATE", [128, 1], f32, kind="ExternalInput")
    ONESM = nc.dram_tensor("ONESM", [128, 128], bf16, kind="ExternalInput")
    OUT = nc.dram_tensor("OUT", [T, DIMS], f32, kind="ExternalOutput")

    QTS = nc.dram_tensor("QTS", [HEADS, HD, T], bf16)    # raw (pre-RoPE) Q^T
    KTS = nc.dram_tensor("KTS", [HEADS, HD, TH], bf16)   # raw K^T (with halo)
    VS = nc.dram_tensor("VS", [TH, DIMS], bf16)         # V natural
    OTS = nc.dram_tensor("OTS", [HEADS, HD, T], f32r)   # normalized O^T

    def tok_tiles(n):
        out, a = [], 0
        while a < n:
            w = min(512, n - a)
            out.append((a, w))
            a += w
        return out

    with tile.TileContext(nc) as tc:
        with tc.tile_pool(name="cst", bufs=1) as cst:
            tri23 = cst.tile([128, 2 * WIN], bf16)
            pgate = cst.tile([128, 1], f32)
            onesm = cst.tile([128, 128], bf16)
            nc.sync.dma_start(tri23[:], TRI23[:])
            nc.sync.dma_start(pgate[:], PGATE[:])
            nc.sync.dma_start(onesm[:], ONESM[:])

            # ---------------- P1: projections ----------------
            if 1 in phases:
              with tc.tile_pool(name="p1", bufs=1) as p1, \
                 tc.tile_pool(name="wp", bufs=10) as wp, \
                 tc.tile_pool(name="st", bufs=8) as st, \
                 tc.tile_pool(name="pp", bufs=8, space="PSUM") as pp:
                ht = p1.tile([128, IB, TH], f32r)
                nc.sync.dma_start(ht[:], HT.rearrange("(ib p) t -> p ib t", p=128))

                # QT / KT: lhsT = W block [128in, 128out], rhs = hT
                for W_, DST, t0, tlen in ((WQ, QTS, WIN, T), (WK, KTS, 0, TH)):
                    for ob in range(HEADS):
                        tts = tok_tiles(tlen)
                        psums = [pp.tile([128, 512], f32, name="pp") for _ in tts]
                        for ib in range(IB):
                            wt = wp.tile([128, 128], f32r, name="w")
                            nc.sync.dma_start(
                                wt[:], W_[ib * 128:(ib + 1) * 128,
                                          ob * 128:(ob + 1) * 128])
                            for ti, (a, w) in enumerate(tts):
                                nc.tensor.matmul(
                                    psums[ti][:, :w], wt[:],
                                    ht[:, ib, t0 + a:t0 + a + w],
                                    start=(ib == 0), stop=(ib == IB - 1))
                        for ti, (a, w) in enumerate(tts):
                            so = st.tile([128, 512], bf16, name="st")
                            nc.scalar.copy(so[:, :w], psums[ti][:, :w])
                            nc.sync.dma_start(DST[ob][:, a:a + w], so[:, :w])

                # V natural: lhsT = hT block [128in, 128tok], rhs = Wv rows
                NTB = TH // 128
                for tb0 in range(0, NTB, 6):
                    tbs = list(range(tb0, min(tb0 + 6, NTB)))
                    for og in range(4):
                        psums = {}
                        for ib in range(IB):
                            wt = wp.tile([128, 512], f32r, name="wv")
                            nc.sync.dma_start(
                                wt[:], WV[ib * 128:(ib + 1) * 128,
                                          og * 512:(og + 1) * 512])
                            for tb in tbs:
                                if ib == 0:
                                    psums[tb] = pp.tile([128, 512], f32, name="pp")
                                nc.tensor.matmul(
                                    psums[tb][:],
                                    ht[:, ib, tb * 128:(tb + 1) * 128], wt[:],
                                    start=(ib == 0), stop=(ib == IB - 1))
                        for tb in tbs:
                            so = st.tile([128, 512], bf16, name="stv")
                            nc.vector.tensor_copy(so[:], psums[tb][:])
                            nc.sync.dma_start(
                                VS[tb * 128:(tb + 1) * 128,
                                   og * 512:(og + 1) * 512], so[:])

            # ---------------- P2: attention ----------------
            if 2 in phases:
              with tc.tile_pool(name="qk", bufs=2) as qk, \
                 tc.tile_pool(name="rt", bufs=1) as rt, \
                 tc.tile_pool(name="tp", bufs=3) as tp, \
                 tc.tile_pool(name="ptp", bufs=2) as ptp, \
                 tc.tile_pool(name="ex", bufs=2) as exp_pool, \
                 tc.tile_pool(name="ob", bufs=2) as obp, \
                 tc.tile_pool(name="ps_s", bufs=4, space="PSUM") as ps_s, \
                 tc.tile_pool(name="ps_d", bufs=2, space="PSUM") as ps_d, \
                 tc.tile_pool(name="ps_o", bufs=2, space="PSUM") as ps_o:
                def rope_load(SRC, c0, roped, which, pos0=None):
                    """Load [128, HEADS, WIN] token window at c0 from SRC
                    (head-major scratch), apply RoPE into `roped` (f32r).
                    pos0: column into COS/SINS (halo coords); default c0.
                    cos/sin slices are DMA-replicated x4 so the DVE ops run
                    on [128, 4*WIN] four-head groups."""
                    if pos0 is None:
                        pos0 = c0
                    raw = rt.tile([128, HEADS, WIN], bf16, name=f"raw{which}")
                    rot = rt.tile([128, HEADS, WIN], bf16, name=f"rot{which}")
                    sl = SRC[:, :, c0:c0 + WIN]
                    nc.sync.dma_start(raw[:], sl.rearrange("h d w -> d h w"))
                    nc.sync.dma_start(rot[0:64], sl[:, 64:128, :].rearrange("h d w -> d h w"))
                    nc.sync.dma_start(rot[64:128], sl[:, 0:64, :].rearrange("h d w -> d h w"))
                    cos4 = tp.tile([128, 4, WIN], f32, name="cos4")
                    sin4 = tp.tile([128, 4, WIN], f32, name="sin4")
                    for g in range(4):
                        nc.sync.dma_start(cos4[:, g], COS[:, pos0:pos0 + WIN])
                        nc.sync.dma_start(sin4[:, g], SINS[:, pos0:pos0 + WIN])
                    for g in range(4):
                        gs = slice(g * 4, (g + 1) * 4)
                        tmp = tp.tile([128, 4, WIN], bf16, name="tmp")
                        nc.vector.tensor_mul(tmp[:], rot[:, gs], sin4[:])
                        nc.vector.tensor_mul(roped[:, gs], raw[:, gs], cos4[:])
                        nc.vector.tensor_add(roped[:, gs], roped[:, gs], tmp[:])

                kt_prev = qk.tile([128, HEADS, WIN], bf16, name="kt")
                rope_load(KTS, 0, kt_prev, "k")
                v_prev = qk.tile([128, 2, DIMS], bf16, name="v")
                nc.sync.dma_start(
                    v_prev[:], VS[0:WIN].rearrange("(tb p) c -> p tb c", p=128))

                for c in range(NC_):
                    kt_cur = qk.tile([128, HEADS, WIN], bf16, name="kt")
                    rope_load(KTS, WIN + c * WIN, kt_cur, "k")
                    v_cur = qk.tile([128, 2, DIMS], bf16, name="v")
                    nc.sync.dma_start(
                        v_cur[:], VS[WIN + c * WIN:WIN + (c + 1) * WIN]
                        .rearrange("(tb p) c -> p tb c", p=128))
                    qt = qk.tile([128, HEADS, WIN], bf16, name="qt")
                    rope_load(QTS, c * WIN, qt, "q", pos0=WIN + c * WIN)

                    kts = [kt_prev, kt_prev, kt_cur, kt_cur]
                    vs = [v_prev, v_prev, v_cur, v_cur]
                    W2 = 2 * WIN
                    for h0 in range(0, HEADS, 2):
                        # per head-pair psums: denominator and O^T share
                        # [128, 512] banks (head h0 left, h0+1 right)
                        pd = ps_d.tile([128, W2], f32, name="pd")
                        po = ps_o.tile([128, W2], f32, name="po")
                        pts2 = []
                        for h in (h0, h0 + 1):
                            # scores: kb0|kb1 pair and kb2|kb3 pair in one bank
                            pts = []
                            for pr in range(2):
                                ps = ps_s.tile([128, W2], f32, name="ps")
                                for kb2 in range(2):
                                    kb = pr * 2 + kb2
                                    nc.tensor.matmul(
                                        ps[:, kb2 * WIN:(kb2 + 1) * WIN],
                                        kts[kb][:, h, (kb % 2) * 128:(kb % 2) * 128 + 128],
                                        qt[:, h], start=True, stop=True)
                                pb = ptp.tile([128, W2], bf16, name=f"pt{pr}")
                                if pr == 0:
                                    if c == 0:
                                        nc.scalar.activation(pb[:], ps[:], AF.Exp,
                                                             bias=pgate[:], scale=ISQ)
                                    else:
                                        nc.scalar.activation(pb[:], ps[:], AF.Exp,
                                                             scale=ISQ)
                                else:
                                    ex = exp_pool.tile([128, W2], bf16, name="ex")
                                    nc.scalar.activation(ex[:], ps[:], AF.Exp,
                                                         scale=ISQ)
                                    nc.vector.tensor_mul(pb[:], ex[:], tri23[:])
                                pts.append(pb)
                            pts2.append(pts)

                        for i, h in enumerate((h0, h0 + 1)):
                            sl = slice(i * WIN, (i + 1) * WIN)
                            for kb in range(4):
                                pb = pts2[i][kb // 2][:, (kb % 2) * WIN:(kb % 2 + 1) * WIN]
                                nc.tensor.matmul(pd[:, sl], onesm[:], pb,
                                                 start=(kb == 0), stop=(kb == 3))
                            for kb in range(4):
                                pb = pts2[i][kb // 2][:, (kb % 2) * WIN:(kb % 2 + 1) * WIN]
                                nc.tensor.matmul(
                                    po[:, sl], vs[kb][:, kb % 2, h * 128:(h + 1) * 128],
                                    pb, start=(kb == 0), stop=(kb == 3))
                        rb = obp.tile([128, W2], f32, name="rb")
                        with nc.allow_low_precision("softmax denominator"):
                            nc.vector.reciprocal(rb[:], pd[:])
                        ot = obp.tile([128, W2], f32r, name="ot")
                        nc.vector.tensor_mul(ot[:], po[:], rb[:])
                        nc.sync.dma_start(OTS[h0][:, c * WIN:(c + 1) * WIN],
                                          ot[:, 0:WIN])
                        nc.sync.dma_start(OTS[h0 + 1][:, c * WIN:(c + 1) * WIN],
                                          ot[:, WIN:W2])
                    kt_prev, v_prev = kt_cur, v_cur

            # ---------------- P3: output projection ----------------
            if 3 in phases:
              with tc.tile_pool(name="p3", bufs=1) as p3, \
                 tc.tile_pool(name="otp", bufs=3) as otp, \
                 tc.tile_pool(name="so3", bufs=6) as so3, \
                 tc.tile_pool(name="pp3", bufs=8, space="PSUM") as pp3:
                wo = p3.tile([128, IB, DIMS], f32r)
                nc.sync.dma_start(wo[:], WO.rearrange("(ib p) d -> p ib d", p=128))
                for tt in range(T // 128):
                    ots = otp.tile([128, HEADS, 128], f32r, name="ots")
                    nc.sync.dma_start(
                        ots[:], OTS[:, :, tt * 128:(tt + 1) * 128]
                        .rearrange("h d w -> d h w"))
                    for nt in range(4):
                        ps = pp3.tile([128, 512], f32, name="pp3")
                        for h in range(HEADS):
                            nc.tensor.matmul(
                                ps[:], ots[:, h], wo[:, h, nt * 512:(nt + 1) * 512],
                                start=(h == 0), stop=(h == HEADS - 1))
                        so = so3.tile([128, 512], f32, name="so")
                        nc.scalar.copy(so[:], ps[:])
                        nc.sync.dma_start(
                            OUT[tt * 128:(tt + 1) * 128,
                                nt * 512:(nt + 1) * 512], so[:])
    return nc


def _host_inputs(hidden_states, Wq, Wk, Wv, Wo, T):
    """Build the 8 per-core input maps."""
    TH = T + WIN
    inv_freq = 1.0 / (THETA ** (np.arange(0, HD, 2, dtype=np.float32) / HD))

    qq = np.arange(WIN)[None, :]
    kk = np.arange(128)[:, None]
    tri23 = np.concatenate([(qq >= kk), (qq >= kk + 128)], 1).astype(ml_dtypes.bfloat16)
    onesm_bf = np.ones((128, 128), ml_dtypes.bfloat16)

    Wq, Wk, Wv, Wo = (np.ascontiguousarray(w, np.float32) for w in (Wq, Wk, Wv, Wo))
    in_maps = []
    for core in range(8):
        b, sh = divmod(core, NSH)
        t0 = sh * T
        hs = np.zeros((TH, DIMS), np.float32)
        lo = max(0, t0 - WIN)
        hs[WIN - (t0 - lo):] = hidden_states[b, lo:t0 + T]
        hT = np.ascontiguousarray(hs.T)

        pos = np.arange(t0 - WIN, t0 + T, dtype=np.float32)
        f = np.outer(inv_freq, pos)                      # [64, TH]
        cos = np.concatenate([np.cos(f), np.cos(f)], 0)  # [128, TH]
        sin = np.sin(f)
        sins = np.concatenate([-sin, sin], 0)
        pg = np.full((128, 1), -1e30 if sh == 0 else 0.0, np.float32)
        in_maps.append({
            "HT": hT, "WQ": Wq, "WK": Wk, "WV": Wv, "WO": Wo,
            "COS": cos.astype(np.float32), "SINS": sins.astype(np.float32),
            "TRI23": tri23, "PGATE": pg, "ONESM": onesm_bf,
        })
    return in_maps


_CACHE = {}


def run(hidden_states, Wq, Wk, Wv, Wo, T=S // NSH, **spmd_kwargs):
    key = T
    if key not in _CACHE:
        nc = bacc.Bacc(None)
        build(nc, T)
        nc.finalize()
        _CACHE[key] = nc
    nc = _CACHE[key]
    in_maps = _host_inputs(hidden_states, Wq, Wk, Wv, Wo, T)
    res = run_bass_kernel_spmd(nc, in_maps, core_ids=list(range(8)), **spmd_kwargs)
    outs = [res.results[i]["OUT"] for i in range(8)]
    full = np.empty((B, NSH * T, DIMS), np.float32)
    for core in range(8):
        b, sh = divmod(core, NSH)
        full[b, sh * T:(sh + 1) * T] = outs[core]
    return full, res


def kernel(hidden_states, Wq, Wk, Wv, Wo):
    out, _ = run(np.asarray(hidden_states), Wq, Wk, Wv, Wo)
    return out



# revision 14
# speedup vs baseline: 1.2158x; 1.2158x over previous
"""Block sliding-window attention on 8 TRN2 NeuronCores.

Sharding: sequence-parallel. 8 shards = (batch b in {0,1}) x (quarter s in
0..3); each core owns 2048 consecutive tokens of one batch plus a 256-token
K/V halo from the previous quarter (zeros + -inf gate for the first quarter).
No collectives: each core computes its tokens' full output rows.

Per-core pipeline (all matmuls bf16: full PE rate):
  P1: K^T/Q^T = W^T @ hiddenT (head-transposed layout, raw), V = hidden @ Wv
      (natural layout), staged through DRAM scratch. hiddenT is streamed in
      5 token-group DMAs so the first matmuls start ~7us in; per head-column
      the 16 weight tiles arrive as one DMA and stay resident.
  P2+P3 fused per 256-token chunk: RoPE on Q/K (rot-half via partition-offset
      DMA reload + pre-signed sin, cos/sin broadcast via stride-0 APs), then
      per head: S^T = K Q^T per 128-key block into one 2-bank PSUM tile,
      single exp on ACT (scale=1/sqrt(128), -1e30 bias gates the no-previous
      case), 0/1 triangular mask multiply on DVE, denominator via DVE
      pre-add + one all-ones matmul (broadcasts across partitions),
      O^T = V^T P^T, normalize with DVE reciprocal; afterwards the chunk's
      256 output rows are projected against SBUF-resident Wo and stored.
      P2 loads ride the Pool queue; stores ride the sync queue.
"""
import sys

try:
    import concourse  # noqa: F401
except ImportError:
    sys.path.insert(0, '/opt/trn_rl_repo')

import ml_dtypes
import numpy as np

import concourse.bacc as bacc
import concourse.mybir as mybir
import concourse.tile as tile
from concourse.bass import broadcast_tensor_aps
from concourse.bass_utils import run_bass_kernel_spmd

f32 = mybir.dt.float32
AF = mybir.ActivationFunctionType
bf16 = mybir.dt.bfloat16

DIMS = 2048
HEADS = 16
HD = 128           # head dim
WIN = 256          # window / chunk
B, S = 2, 8192
NSH = 4            # seq shards per batch
THETA = 10000.0
ISQ = float(1.0 / np.sqrt(HD))
IB = DIMS // 128   # 16 input-dim blocks


def tok_tiles(n):
    out, a = [], 0
    while a < n:
        w = min(512, n - a)
        out.append((a, w))
        a += w
    return out


def build(nc, T):
    """Emit the per-core program. T = local tokens (multiple of 512)."""
    TH = T + WIN                      # with halo
    NC_ = T // WIN                    # chunks
    HT = nc.dram_tensor("HT", [DIMS, TH], bf16, kind="ExternalInput")
    WQ = nc.dram_tensor("WQ", [DIMS, DIMS], bf16, kind="ExternalInput")
    WK = nc.dram_tensor("WK", [DIMS, DIMS], bf16, kind="ExternalInput")
    WV = nc.dram_tensor("WV", [DIMS, DIMS], bf16, kind="ExternalInput")
    WO = nc.dram_tensor("WO", [DIMS, DIMS], bf16, kind="ExternalInput")
    COS = nc.dram_tensor("COS", [HD, TH], bf16, kind="ExternalInput")
    SINS = nc.dram_tensor("SINS", [HD, TH], bf16, kind="ExternalInput")
    TRI23 = nc.dram_tensor("TRI23", [128, 2, WIN], bf16, kind="ExternalInput")
    PGATE = nc.dram_tensor("PGATE", [128, 1], f32, kind="ExternalInput")
    ONESM = nc.dram_tensor("ONESM", [128, 128], bf16, kind="ExternalInput")
    OUT = nc.dram_tensor("OUT", [T, DIMS], f32, kind="ExternalOutput")

    QTS = nc.dram_tensor("QTS", [HEADS, HD, T], bf16)    # raw (pre-RoPE) Q^T
    KTS = nc.dram_tensor("KTS", [HEADS, HD, TH], bf16)   # raw K^T (with halo)
    VS = nc.dram_tensor("VS", [TH, DIMS], bf16)          # V natural

    with tile.TileContext(nc) as tc:
        with tc.tile_pool(name="cst", bufs=1) as cst, \
             tc.tile_pool(name="qk", bufs=2) as qk, \
             tc.tile_pool(name="rt", bufs=2) as rt:
            tri23 = cst.tile([128, 2, WIN], bf16)
            pgate = cst.tile([128, 1], f32)
            onesm = cst.tile([128, 128], bf16)
            cosb = cst.tile([128, 1, TH], bf16)
            sinb = cst.tile([128, 1, TH], bf16)
            nc.gpsimd.dma_start(tri23[:], TRI23[:])
            nc.gpsimd.dma_start(pgate[:], PGATE[:])
            nc.gpsimd.dma_start(onesm[:], ONESM[:])
            nc.gpsimd.dma_start(cosb[:, 0], COS[:])
            nc.gpsimd.dma_start(sinb[:, 0], SINS[:])

            # ---------------- P1: projections ----------------
            with tc.tile_pool(name="p1", bufs=1) as p1, \
                 tc.tile_pool(name="wp", bufs=2) as wp, \
                 tc.tile_pool(name="st", bufs=2) as st, \
                 tc.tile_pool(name="pp", bufs=4, space="PSUM") as pp:
                def load_wt(W_, ob):
                    wt = wp.tile([128, IB, 128], bf16, name="w")
                    nc.sync.dma_start(
                        wt[:], W_[:, ob * 128:(ob + 1) * 128]
                        .rearrange("(ib p) o -> p ib o", p=128))
                    return wt

                # first weight tile ahead of the big hidden-state load so
                # the PE can start as soon as token group 0 lands
                wt_next = load_wt(WK, 0)
                ht = p1.tile([128, IB, TH], bf16)
                for a, w in tok_tiles(TH):
                    nc.sync.dma_start(
                        ht[:, :, a:a + w],
                        HT[:, a:a + w].rearrange("(ib p) t -> p ib t", p=128))

                # K^T / Q^T: lhsT = W column block [128in, 128out]
                seq = [(WK, KTS, 0, TH, ob) for ob in range(HEADS)] + \
                      [(WQ, QTS, WIN, T, ob) for ob in range(HEADS)]
                for idx, (W_, DST, t0, tlen, ob) in enumerate(seq):
                    wt = wt_next
                    if idx + 1 < len(seq):
                        nw, _, _, _, nob = seq[idx + 1]
                        wt_next = load_wt(nw, nob)
                    stg = st.tile([128, TH], bf16, name="stg", bufs=1)
                    for a, w in tok_tiles(tlen):
                        ps = pp.tile([128, 512], f32, name="pp")
                        for ib in range(IB):
                            nc.tensor.matmul(
                                ps[:, :w], wt[:, ib],
                                ht[:, ib, t0 + a:t0 + a + w],
                                start=(ib == 0), stop=(ib == IB - 1))
                        nc.scalar.copy(stg[:, a:a + w], ps[:, :w])
                    nc.sync.dma_start(DST[ob][:, 0:tlen], stg[:, 0:tlen])

                # V natural: lhsT = hT block [128in, 128tok], rhs = Wv rows
                def load_wv(og):
                    wv = wp.tile([128, IB, 512], bf16, name="wv")
                    nc.sync.dma_start(
                        wv[:], WV[:, og * 512:(og + 1) * 512]
                        .rearrange("(ib p) d -> p ib d", p=128))
                    return wv

                wv_next = load_wv(0)
                for og in range(4):
                    wv = wv_next
                    if og + 1 < 4:
                        wv_next = load_wv(og + 1)
                    for tb in range(TH // 128):
                        ps = pp.tile([128, 512], f32, name="pp")
                        for ib in range(IB):
                            nc.tensor.matmul(
                                ps[:], ht[:, ib, tb * 128:(tb + 1) * 128],
                                wv[:, ib, :],
                                start=(ib == 0), stop=(ib == IB - 1))
                        stgv = st.tile([128, 512], bf16, name="stgv")
                        nc.scalar.copy(stgv[:], ps[:])
                        nc.sync.dma_start(
                            VS[tb * 128:(tb + 1) * 128,
                               og * 512:(og + 1) * 512], stgv[:])

            # ---------------- P2 + P3 fused ----------------
            with tc.tile_pool(name="wop", bufs=1) as wop, \
                 tc.tile_pool(name="pbp", bufs=5) as pbp, \
                 tc.tile_pool(name="pad", bufs=2) as padp, \
                 tc.tile_pool(name="pad2", bufs=4) as padp2, \
                 tc.tile_pool(name="ob", bufs=2) as obp, \
                 tc.tile_pool(name="otp", bufs=10) as otp, \
                 tc.tile_pool(name="st3", bufs=2) as st3, \
                 tc.tile_pool(name="ps_s", bufs=2, space="PSUM") as ps_s, \
                 tc.tile_pool(name="ps_po", bufs=1, space="PSUM") as ps_po, \
                 tc.tile_pool(name="ps_p", bufs=2, space="PSUM") as ps_p:

                def rope_issue(SRC, c0):
                    """Issue the raw + rotate-half loads for a 256-token
                    window at c0 of SRC (head-major scratch)."""
                    raw = rt.tile([128, HEADS, WIN], bf16, name="raw")
                    rot = rt.tile([128, HEADS, WIN], bf16, name="rot", bufs=1)
                    sl = SRC[:, :, c0:c0 + WIN]
                    nc.gpsimd.dma_start(raw[:], sl.rearrange("h d w -> d h w"))
                    nc.gpsimd.dma_start(
                        rot[0:64], sl[:, 64:128, :].rearrange("h d w -> d h w"))
                    nc.gpsimd.dma_start(
                        rot[64:128], sl[:, 0:64, :].rearrange("h d w -> d h w"))
                    return raw, rot

                def rope_finish(raw_rot, which, pos0):
                    """RoPE on DVE: roped = raw*cos + rot(-half)*sin."""
                    raw, rot = raw_rot
                    roped = qk.tile([128, HEADS, WIN], bf16, name=which)
                    cs = cosb[:, :, pos0:pos0 + WIN]
                    sn = sinb[:, :, pos0:pos0 + WIN]
                    nc.vector.tensor_mul(
                        rot[:], *broadcast_tensor_aps(rot[:], sn))
                    nc.vector.tensor_mul(
                        roped[:], *broadcast_tensor_aps(raw[:], cs))
                    nc.vector.tensor_add(roped[:], roped[:], rot[:])
                    return roped

                def v_issue(w0):
                    v = qk.tile([128, 2, DIMS], bf16, name="v")
                    nc.gpsimd.dma_start(
                        v[:], VS[w0:w0 + WIN].rearrange("(tb p) c -> p tb c",
                                                        p=128))
                    return v

                # prologue: halo window + chunk-0 tiles. Rope loads first
                # (K/Q scratch is ready mid-P1, so these drain early); the
                # V loads and the big Wo load go behind them in the queue.
                kt_prev = rope_finish(rope_issue(KTS, 0), "kt", 0)
                kt_cur = rope_finish(rope_issue(KTS, WIN), "kt", WIN)
                qt = rope_finish(rope_issue(QTS, 0), "qt", WIN)
                v_prev = v_issue(0)
                v_cur = v_issue(WIN)
                wo = wop.tile([128, IB, DIMS], bf16)
                for nt in range(4):
                    nc.gpsimd.dma_start(
                        wo[:, :, nt * 512:(nt + 1) * 512],
                        WO[:, nt * 512:(nt + 1) * 512]
                        .rearrange("(ib p) d -> p ib d", p=128))

                W2 = 2 * WIN
                for c in range(NC_):
                    if c + 1 < NC_:
                        kt_next_rr = rope_issue(KTS, WIN + (c + 1) * WIN)
                        qt_next_rr = rope_issue(QTS, (c + 1) * WIN)
                        v_next = v_issue(WIN + (c + 1) * WIN)

                    kts = [kt_prev, kt_prev, kt_cur, kt_cur]
                    vs = [v_prev, v_prev, v_cur, v_cur]
                    ots_c = []
                    pend = None  # (h0, [(h, pb, pa2) x2]) awaiting den+O

                    def den_o(pair):
                        pod = ps_po.tile([128, 4, WIN], f32, name="pod")
                        for i, (h, pb, pa2) in enumerate(pair):
                            nc.tensor.matmul(pod[:, 2 + i], onesm[:], pa2[:],
                                             start=True, stop=True)
                            for kb in range(4):
                                nc.tensor.matmul(
                                    pod[:, i],
                                    vs[kb][:, kb % 2, h * 128:(h + 1) * 128],
                                    pb[:, kb], start=(kb == 0), stop=(kb == 3))
                        rb = obp.tile([128, 2, WIN], f32, name="rb")
                        with nc.allow_low_precision("softmax denominator"):
                            nc.vector.reciprocal(rb[:], pod[:, 2:4])
                        ot = otp.tile([128, 2, WIN], bf16, name="ot")
                        nc.vector.tensor_mul(ot[:], pod[:, 0:2], rb[:])
                        ots_c.append(ot)

                    for h0 in range(0, HEADS, 2):
                        pair = []
                        for h in (h0, h0 + 1):
                            # scores for all 4 key blocks in one 2-bank tile
                            ps = ps_s.tile([128, 4, WIN], f32, name="ps")
                            for kb in range(4):
                                nc.tensor.matmul(
                                    ps[:, kb],
                                    kts[kb][:, h,
                                            (kb % 2) * 128:(kb % 2) * 128 + 128],
                                    qt[:, h], start=True, stop=True)
                            pb = pbp.tile([128, 4, WIN], bf16, name="pb")
                            if c == 0:
                                nc.scalar.activation(
                                    pb[:, 0:2], ps[:, 0:2], AF.Exp,
                                    bias=pgate[:], scale=ISQ)
                                nc.scalar.activation(
                                    pb[:, 2:4], ps[:, 2:4], AF.Exp, scale=ISQ)
                            else:
                                nc.scalar.activation(
                                    pb[:], ps[:], AF.Exp, scale=ISQ)
                            nc.vector.tensor_mul(
                                pb[:, 2:4], pb[:, 2:4], tri23[:])
                            # denominator pre-add on DVE
                            pa = padp.tile([128, 2, WIN], bf16, name="pa")
                            nc.vector.tensor_add(pa[:], pb[:, 0:2], pb[:, 2:4])
                            pa2 = padp2.tile([128, WIN], bf16, name="pa2")
                            nc.vector.tensor_add(pa2[:], pa[:, 0], pa[:, 1])
                            pair.append((h, pb, pa2))
                        if pend is not None:
                            den_o(pend)
                        pend = pair
                    den_o(pend)

                    # next chunk's RoPE runs on DVE under P3's matmuls
                    if c + 1 < NC_:
                        kt_next = rope_finish(kt_next_rr, "kt",
                                              WIN + (c + 1) * WIN)
                        qt_next = rope_finish(qt_next_rr, "qt",
                                              WIN + (c + 1) * WIN)

                    # P3: project this chunk's 256 output rows against Wo
                    for tt in range(2):
                        stg = st3.tile([128, DIMS], f32, name="st3")
                        for nt in range(4):
                            ps = ps_p.tile([128, 512], f32, name="pp3")
                            for h in range(HEADS):
                                nc.tensor.matmul(
                                    ps[:],
                                    ots_c[h // 2][:, h % 2,
                                                  tt * 128:(tt + 1) * 128],
                                    wo[:, h, nt * 512:(nt + 1) * 512],
                                    start=(h == 0), stop=(h == HEADS - 1))
                            nc.scalar.copy(
                                stg[:, nt * 512:(nt + 1) * 512], ps[:])
                        nc.sync.dma_start(
                            OUT[c * WIN + tt * 128:c * WIN + (tt + 1) * 128, :],
                            stg[:])
                    if c + 1 < NC_:
                        kt_prev, v_prev = kt_cur, v_cur
                        kt_cur, v_cur, qt = kt_next, v_next, qt_next
    return nc


def _host_inputs(hidden_states, Wq, Wk, Wv, Wo, T):
    """Build the 8 per-core input maps."""
    TH = T + WIN
    inv_freq = 1.0 / (THETA ** (np.arange(0, HD, 2, dtype=np.float32) / HD))

    qq = np.arange(WIN)[None, :]
    kk = np.arange(128)[:, None]
    tri23 = np.stack([(qq >= kk), (qq >= kk + 128)], 1).astype(ml_dtypes.bfloat16)
    onesm_bf = np.ones((128, 128), ml_dtypes.bfloat16)

    Wq, Wk, Wv, Wo = (np.asarray(w, np.float32).astype(ml_dtypes.bfloat16)
                      for w in (Wq, Wk, Wv, Wo))
    in_maps = []
    for core in range(8):
        b, sh = divmod(core, NSH)
        t0 = sh * T
        hs = np.zeros((TH, DIMS), np.float32)
        lo = max(0, t0 - WIN)
        hs[WIN - (t0 - lo):] = hidden_states[b, lo:t0 + T]
        hT = np.ascontiguousarray(hs.T).astype(ml_dtypes.bfloat16)

        pos = np.arange(t0 - WIN, t0 + T, dtype=np.float32)
        f = np.outer(inv_freq, pos)                      # [64, TH]
        cos = np.concatenate([np.cos(f), np.cos(f)], 0)  # [128, TH]
        sin = np.sin(f)
        sins = np.concatenate([-sin, sin], 0)
        pg = np.full((128, 1), -1e30 if sh == 0 else 0.0, np.float32)
        in_maps.append({
            "HT": hT, "WQ": Wq, "WK": Wk, "WV": Wv, "WO": Wo,
            "COS": cos.astype(ml_dtypes.bfloat16),
            "SINS": sins.astype(ml_dtypes.bfloat16),
            "TRI23": tri23, "PGATE": pg, "ONESM": onesm_bf,
        })
    return in_maps


_CACHE = {}


def run(hidden_states, Wq, Wk, Wv, Wo, T=S // NSH, **spmd_kwargs):
    key = T
    if key not in _CACHE:
        nc = bacc.Bacc(None)
        build(nc, T)
        nc.finalize()
        _CACHE[key] = nc
    nc = _CACHE[key]
    in_maps = _host_inputs(hidden_states, Wq, Wk, Wv, Wo, T)
    res = run_bass_kernel_spmd(nc, in_maps, core_ids=list(range(8)), **spmd_kwargs)
    outs = [res.results[i]["OUT"] for i in range(8)]
    full = np.empty((B, NSH * T, DIMS), np.float32)
    for core in range(8):
        b, sh = divmod(core, NSH)
        full[b, sh * T:(sh + 1) * T] = outs[core]
    return full, res


def kernel(hidden_states, Wq, Wk, Wv, Wo):
    out, _ = run(np.asarray(hidden_states), Wq, Wk, Wv, Wo)
    return out


# revision 20
# speedup vs baseline: 1.2610x; 1.0372x over previous
"""Block sliding-window attention on 8 TRN2 NeuronCores.

Sharding: sequence-parallel. 8 shards = (batch b in {0,1}) x (quarter s in
0..3); each core owns 2048 consecutive tokens of one batch plus a 256-token
K/V halo from the previous quarter (zeros + -inf gate for the first quarter).
No collectives: each core computes its tokens' full output rows.

Per-core pipeline (all matmuls bf16: full PE rate):
  P1: K^T/Q^T = W^T @ hiddenT (head-transposed layout, raw), V = hidden @ Wv
      (natural layout), staged through DRAM scratch. hiddenT is streamed in
      5 token-group DMAs so the first matmuls start ~7us in; per head-column
      the 16 weight tiles arrive as one DMA and stay resident.
  P2+P3 fused per 256-token chunk: RoPE on Q/K (rot-half via partition-offset
      DMA reload + pre-signed sin, cos/sin broadcast via stride-0 APs), then
      per head: S^T = K Q^T per 128-key block into one 2-bank PSUM tile,
      single exp on ACT (scale=1/sqrt(128), -1e30 bias gates the no-previous
      case), 0/1 triangular mask multiply on DVE, denominator via DVE
      pre-add + one all-ones matmul (broadcasts across partitions),
      O^T = V^T P^T, normalize with DVE reciprocal; afterwards the chunk's
      256 output rows are projected against SBUF-resident Wo and stored.
      P2 loads ride the Pool queue; stores ride the sync queue.
"""
import sys

try:
    import concourse  # noqa: F401
except ImportError:
    sys.path.insert(0, '/opt/trn_rl_repo')

import ml_dtypes
import numpy as np

import concourse.bacc as bacc
import concourse.mybir as mybir
import concourse.tile as tile
from concourse.bass_utils import run_bass_kernel_spmd

f32 = mybir.dt.float32
AF = mybir.ActivationFunctionType
bf16 = mybir.dt.bfloat16

DIMS = 2048
HEADS = 16
HD = 128           # head dim
WIN = 256          # window / chunk
B, S = 2, 8192
NSH = 4            # seq shards per batch
THETA = 10000.0
ISQ = float(1.0 / np.sqrt(HD))
IB = DIMS // 128   # 16 input-dim blocks


def tok_tiles(n):
    out, a = [], 0
    while a < n:
        w = min(512, n - a)
        out.append((a, w))
        a += w
    return out


def build(nc, T):
    """Emit the per-core program. T = local tokens (multiple of 512)."""
    TH = T + WIN                      # with halo
    NC_ = T // WIN                    # chunks
    HT = nc.dram_tensor("HT", [DIMS, TH], bf16, kind="ExternalInput")
    WQ = nc.dram_tensor("WQ", [DIMS, DIMS], bf16, kind="ExternalInput")
    WK = nc.dram_tensor("WK", [DIMS, DIMS], bf16, kind="ExternalInput")
    WV = nc.dram_tensor("WV", [DIMS, DIMS], bf16, kind="ExternalInput")
    WO = nc.dram_tensor("WO", [DIMS, DIMS], bf16, kind="ExternalInput")
    COS = nc.dram_tensor("COS", [HD, TH], bf16, kind="ExternalInput")
    SINS = nc.dram_tensor("SINS", [HD, TH], bf16, kind="ExternalInput")
    TRI23 = nc.dram_tensor("TRI23", [128, 2, WIN], bf16, kind="ExternalInput")
    PGATE = nc.dram_tensor("PGATE", [128, 1], f32, kind="ExternalInput")
    ONESM = nc.dram_tensor("ONESM", [128, 128], bf16, kind="ExternalInput")
    OUT = nc.dram_tensor("OUT", [T, DIMS], f32, kind="ExternalOutput")

    QTS = nc.dram_tensor("QTS", [HEADS, HD, T], bf16)    # raw (pre-RoPE) Q^T
    KTS = nc.dram_tensor("KTS", [HEADS, HD, TH], bf16)   # raw K^T (with halo)
    VS = nc.dram_tensor("VS", [TH, DIMS], bf16)          # V natural

    with tile.TileContext(nc) as tc:
        with tc.tile_pool(name="cst", bufs=1) as cst, \
             tc.tile_pool(name="qk", bufs=2) as qk:
            tri23 = cst.tile([128, 2, WIN], bf16)
            pgate = cst.tile([128, 1], f32)
            onesm = cst.tile([128, 128], bf16)
            cosb = cst.tile([128, 1, TH], bf16)
            sinb = cst.tile([128, 1, TH], bf16)
            nc.gpsimd.dma_start(tri23[:], TRI23[:])
            nc.gpsimd.dma_start(pgate[:], PGATE[:])
            nc.gpsimd.dma_start(onesm[:], ONESM[:])
            nc.gpsimd.dma_start(cosb[:, 0], COS[:])
            nc.gpsimd.dma_start(sinb[:, 0], SINS[:])

            # ---------------- P1: projections ----------------
            with tc.tile_pool(name="p1", bufs=1) as p1, \
                 tc.tile_pool(name="wp", bufs=2) as wp, \
                 tc.tile_pool(name="st", bufs=2) as st, \
                 tc.tile_pool(name="pp", bufs=4, space="PSUM") as pp:
                def load_wt(W_, ob):
                    wt = wp.tile([128, IB, 128], bf16, name="w")
                    nc.sync.dma_start(
                        wt[:], W_[:, ob * 128:(ob + 1) * 128]
                        .rearrange("(ib p) o -> p ib o", p=128))
                    return wt

                # first weight tile ahead of the big hidden-state load so
                # the PE can start as soon as token group 0 lands
                wt_next = load_wt(WK, 0)
                ht = p1.tile([128, IB, TH], bf16)
                for a, w in tok_tiles(TH):
                    nc.sync.dma_start(
                        ht[:, :, a:a + w],
                        HT[:, a:a + w].rearrange("(ib p) t -> p ib t", p=128))

                # K^T / Q^T: lhsT = W column block [128in, 128out].
                # RoPE is applied here, once per head row: rotate-half via
                # SBUF->SBUF partition-swap DMA, then 3 in-place DVE ops
                # (DVE is otherwise idle in P1). pos0 = column into COS/SINS.
                seq = [(WK, KTS, 0, TH, ob) for ob in range(HEADS)] + \
                      [(WQ, QTS, WIN, T, ob) for ob in range(HEADS)]
                for idx, (W_, DST, t0, tlen, ob) in enumerate(seq):
                    wt = wt_next
                    if idx + 1 < len(seq):
                        nw, _, _, _, nob = seq[idx + 1]
                        wt_next = load_wt(nw, nob)
                    stg = st.tile([128, TH], bf16, name="stg")
                    for a, w in tok_tiles(tlen):
                        ps = pp.tile([128, 512], f32, name="pp")
                        for ib in range(IB):
                            nc.tensor.matmul(
                                ps[:, :w], wt[:, ib],
                                ht[:, ib, t0 + a:t0 + a + w],
                                start=(ib == 0), stop=(ib == IB - 1))
                        nc.scalar.copy(stg[:, a:a + w], ps[:, :w])
                    rot = st.tile([128, TH], bf16, name="rot")
                    nc.sync.dma_start(rot[0:64, 0:tlen], stg[64:128, 0:tlen])
                    nc.sync.dma_start(rot[64:128, 0:tlen], stg[0:64, 0:tlen])
                    cs = cosb[:, 0, t0:t0 + tlen]
                    sn = sinb[:, 0, t0:t0 + tlen]
                    nc.vector.tensor_mul(rot[:, 0:tlen], rot[:, 0:tlen], sn)
                    nc.vector.tensor_mul(stg[:, 0:tlen], stg[:, 0:tlen], cs)
                    nc.vector.tensor_add(stg[:, 0:tlen], stg[:, 0:tlen],
                                         rot[:, 0:tlen])
                    nc.sync.dma_start(DST[ob][:, 0:tlen], stg[:, 0:tlen])

                # V natural: lhsT = hT block [128in, 128tok], rhs = Wv rows
                def load_wv(og):
                    wv = wp.tile([128, IB, 512], bf16, name="wv")
                    nc.sync.dma_start(
                        wv[:], WV[:, og * 512:(og + 1) * 512]
                        .rearrange("(ib p) d -> p ib d", p=128))
                    return wv

                wv_next = load_wv(0)
                for og in range(4):
                    wv = wv_next
                    if og + 1 < 4:
                        wv_next = load_wv(og + 1)
                    for tb in range(TH // 128):
                        ps = pp.tile([128, 512], f32, name="pp")
                        for ib in range(IB):
                            nc.tensor.matmul(
                                ps[:], ht[:, ib, tb * 128:(tb + 1) * 128],
                                wv[:, ib, :],
                                start=(ib == 0), stop=(ib == IB - 1))
                        stgv = st.tile([128, 512], bf16, name="stgv")
                        nc.scalar.copy(stgv[:], ps[:])
                        nc.sync.dma_start(
                            VS[tb * 128:(tb + 1) * 128,
                               og * 512:(og + 1) * 512], stgv[:])

            # ---------------- P2 + P3 fused ----------------
            with tc.tile_pool(name="wop", bufs=1) as wop, \
                 tc.tile_pool(name="pbp", bufs=5) as pbp, \
                 tc.tile_pool(name="pad", bufs=2) as padp, \
                 tc.tile_pool(name="pad2", bufs=4) as padp2, \
                 tc.tile_pool(name="ob", bufs=2) as obp, \
                 tc.tile_pool(name="otp", bufs=10) as otp, \
                 tc.tile_pool(name="st3", bufs=2) as st3, \
                 tc.tile_pool(name="ps_s", bufs=2, space="PSUM") as ps_s, \
                 tc.tile_pool(name="ps_po", bufs=1, space="PSUM") as ps_po, \
                 tc.tile_pool(name="ps_p", bufs=2, space="PSUM") as ps_p:

                def kq_issue(SRC, c0, which):
                    t = qk.tile([128, HEADS, WIN], bf16, name=which)
                    nc.gpsimd.dma_start(
                        t[:], SRC[:, :, c0:c0 + WIN]
                        .rearrange("h d w -> d h w"))
                    return t

                def v_issue(w0):
                    v = qk.tile([128, 2, DIMS], bf16, name="v")
                    nc.gpsimd.dma_start(
                        v[:], VS[w0:w0 + WIN].rearrange("(tb p) c -> p tb c",
                                                        p=128))
                    return v

                # prologue: halo window + chunk-0 tiles. K/Q loads first
                # (their scratch is ready mid-P1, so these drain early); the
                # V loads and the big Wo load go behind them in the queue.
                kt_prev = kq_issue(KTS, 0, "kt")
                kt_cur = kq_issue(KTS, WIN, "kt")
                qt = kq_issue(QTS, 0, "qt")
                v_prev = v_issue(0)
                v_cur = v_issue(WIN)
                wo = wop.tile([128, IB, DIMS], bf16)
                for nt in range(4):
                    nc.gpsimd.dma_start(
                        wo[:, :, nt * 512:(nt + 1) * 512],
                        WO[:, nt * 512:(nt + 1) * 512]
                        .rearrange("(ib p) d -> p ib d", p=128))

                W2 = 2 * WIN
                for c in range(NC_):
                    if c + 1 < NC_:
                        kt_next = kq_issue(KTS, WIN + (c + 1) * WIN, "kt")
                        qt_next = kq_issue(QTS, (c + 1) * WIN, "qt")
                        v_next = v_issue(WIN + (c + 1) * WIN)

                    kts = [kt_prev, kt_prev, kt_cur, kt_cur]
                    vs = [v_prev, v_prev, v_cur, v_cur]
                    ots_c = []
                    pend = None  # (h0, [(h, pb, pa2) x2]) awaiting den+O

                    def den_o(pair):
                        pod = ps_po.tile([128, 4, WIN], f32, name="pod")
                        for i, (h, pb, pa2) in enumerate(pair):
                            nc.tensor.matmul(pod[:, 2 + i], onesm[:], pa2[:],
                                             start=True, stop=True)
                            for kb in range(4):
                                nc.tensor.matmul(
                                    pod[:, i],
                                    vs[kb][:, kb % 2, h * 128:(h + 1) * 128],
                                    pb[:, kb], start=(kb == 0), stop=(kb == 3))
                        rb = obp.tile([128, 2, WIN], f32, name="rb")
                        with nc.allow_low_precision("softmax denominator"):
                            nc.vector.reciprocal(rb[:], pod[:, 2:4])
                        ot = otp.tile([128, 2, WIN], bf16, name="ot")
                        nc.vector.tensor_mul(ot[:], pod[:, 0:2], rb[:])
                        ots_c.append(ot)

                    for h0 in range(0, HEADS, 2):
                        pair = []
                        for h in (h0, h0 + 1):
                            # scores for all 4 key blocks in one 2-bank tile
                            ps = ps_s.tile([128, 4, WIN], f32, name="ps")
                            for kb in range(4):
                                nc.tensor.matmul(
                                    ps[:, kb],
                                    kts[kb][:, h,
                                            (kb % 2) * 128:(kb % 2) * 128 + 128],
                                    qt[:, h], start=True, stop=True)
                            pb = pbp.tile([128, 4, WIN], bf16, name="pb")
                            if c == 0:
                                nc.scalar.activation(
                                    pb[:, 0:2], ps[:, 0:2], AF.Exp,
                                    bias=pgate[:], scale=ISQ)
                                nc.scalar.activation(
                                    pb[:, 2:4], ps[:, 2:4], AF.Exp, scale=ISQ)
                            else:
                                nc.scalar.activation(
                                    pb[:], ps[:], AF.Exp, scale=ISQ)
                            nc.vector.tensor_mul(
                                pb[:, 2:4], pb[:, 2:4], tri23[:])
                            # denominator pre-add on DVE
                            pa = padp.tile([128, 2, WIN], bf16, name="pa")
                            nc.vector.tensor_add(pa[:], pb[:, 0:2], pb[:, 2:4])
                            pa2 = padp2.tile([128, WIN], bf16, name="pa2")
                            nc.vector.tensor_add(pa2[:], pa[:, 0], pa[:, 1])
                            pair.append((h, pb, pa2))
                        if pend is not None:
                            den_o(pend)
                        pend = pair
                    den_o(pend)

                    # P3: project this chunk's 256 output rows against Wo
                    for tt in range(2):
                        stg = st3.tile([128, DIMS], f32, name="st3")
                        for nt in range(4):
                            ps = ps_p.tile([128, 512], f32, name="pp3")
                            for h in range(HEADS):
                                nc.tensor.matmul(
                                    ps[:],
                                    ots_c[h // 2][:, h % 2,
                                                  tt * 128:(tt + 1) * 128],
                                    wo[:, h, nt * 512:(nt + 1) * 512],
                                    start=(h == 0), stop=(h == HEADS - 1))
                            nc.scalar.copy(
                                stg[:, nt * 512:(nt + 1) * 512], ps[:])
                        nc.sync.dma_start(
                            OUT[c * WIN + tt * 128:c * WIN + (tt + 1) * 128, :],
                            stg[:])
                    if c + 1 < NC_:
                        kt_prev, v_prev = kt_cur, v_cur
                        kt_cur, v_cur, qt = kt_next, v_next, qt_next
    return nc


def _host_inputs(hidden_states, Wq, Wk, Wv, Wo, T):
    """Build the 8 per-core input maps."""
    TH = T + WIN
    inv_freq = 1.0 / (THETA ** (np.arange(0, HD, 2, dtype=np.float32) / HD))

    qq = np.arange(WIN)[None, :]
    kk = np.arange(128)[:, None]
    tri23 = np.stack([(qq >= kk), (qq >= kk + 128)], 1).astype(ml_dtypes.bfloat16)
    onesm_bf = np.ones((128, 128), ml_dtypes.bfloat16)

    Wq, Wk, Wv, Wo = (np.asarray(w, np.float32).astype(ml_dtypes.bfloat16)
                      for w in (Wq, Wk, Wv, Wo))
    in_maps = []
    for core in range(8):
        b, sh = divmod(core, NSH)
        t0 = sh * T
        hs = np.zeros((TH, DIMS), np.float32)
        lo = max(0, t0 - WIN)
        hs[WIN - (t0 - lo):] = hidden_states[b, lo:t0 + T]
        hT = np.ascontiguousarray(hs.T).astype(ml_dtypes.bfloat16)

        pos = np.arange(t0 - WIN, t0 + T, dtype=np.float32)
        f = np.outer(inv_freq, pos)                      # [64, TH]
        cos = np.concatenate([np.cos(f), np.cos(f)], 0)  # [128, TH]
        sin = np.sin(f)
        sins = np.concatenate([-sin, sin], 0)
        pg = np.full((128, 1), -1e30 if sh == 0 else 0.0, np.float32)
        in_maps.append({
            "HT": hT, "WQ": Wq, "WK": Wk, "WV": Wv, "WO": Wo,
            "COS": cos.astype(ml_dtypes.bfloat16),
            "SINS": sins.astype(ml_dtypes.bfloat16),
            "TRI23": tri23, "PGATE": pg, "ONESM": onesm_bf,
        })
    return in_maps


_CACHE = {}


def run(hidden_states, Wq, Wk, Wv, Wo, T=S // NSH, **spmd_kwargs):
    key = T
    if key not in _CACHE:
        nc = bacc.Bacc(None)
        build(nc, T)
        nc.finalize()
        _CACHE[key] = nc
    nc = _CACHE[key]
    in_maps = _host_inputs(hidden_states, Wq, Wk, Wv, Wo, T)
    res = run_bass_kernel_spmd(nc, in_maps, core_ids=list(range(8)), **spmd_kwargs)
    outs = [res.results[i]["OUT"] for i in range(8)]
    full = np.empty((B, NSH * T, DIMS), np.float32)
    for core in range(8):
        b, sh = divmod(core, NSH)
        full[b, sh * T:(sh + 1) * T] = outs[core]
    return full, res


def kernel(hidden_states, Wq, Wk, Wv, Wo):
    out, _ = run(np.asarray(hidden_states), Wq, Wk, Wv, Wo)
    return out


# revision 30
# speedup vs baseline: 1.2747x; 1.0109x over previous
"""Block sliding-window attention on 8 TRN2 NeuronCores.

Sharding: sequence-parallel. 8 shards = (batch b in {0,1}) x (quarter s in
0..3); each core owns 2048 consecutive tokens of one batch plus a 256-token
K/V halo from the previous quarter (zeros + -inf gate for the first quarter).
No collectives: each core computes its tokens' full output rows.

Per-core pipeline (all matmuls bf16: full PE rate):
  P1: K^T/Q^T = W^T @ hiddenT (head-transposed layout, raw), V = hidden @ Wv
      (natural layout), staged through DRAM scratch. hiddenT is streamed in
      5 token-group DMAs so the first matmuls start ~7us in; per head-column
      the 16 weight tiles arrive as one DMA and stay resident.
  P2+P3 fused per 256-token chunk: RoPE on Q/K (rot-half via partition-offset
      DMA reload + pre-signed sin, cos/sin broadcast via stride-0 APs), then
      per head: S^T = K Q^T per 128-key block into one 2-bank PSUM tile,
      single exp on ACT (scale=1/sqrt(128), -1e30 bias gates the no-previous
      case), 0/1 triangular mask multiply on DVE, denominator via DVE
      pre-add + one all-ones matmul (broadcasts across partitions),
      O^T = V^T P^T, normalize with DVE reciprocal; afterwards the chunk's
      256 output rows are projected against SBUF-resident Wo and stored.
      P2 loads ride the Pool queue; stores ride the sync queue.
"""
import sys

try:
    import concourse  # noqa: F401
except ImportError:
    sys.path.insert(0, '/opt/trn_rl_repo')

import ml_dtypes
import numpy as np

import concourse.bacc as bacc
import concourse.mybir as mybir
import concourse.tile as tile
from concourse.bass_utils import run_bass_kernel_spmd

f32 = mybir.dt.float32
AF = mybir.ActivationFunctionType
bf16 = mybir.dt.bfloat16

DIMS = 2048
HEADS = 16
HD = 128           # head dim
WIN = 256          # window / chunk
B, S = 2, 8192
NSH = 4            # seq shards per batch
THETA = 10000.0
ISQ = float(1.0 / np.sqrt(HD))
IB = DIMS // 128   # 16 input-dim blocks


def tok_tiles(n):
    out, a = [], 0
    while a < n:
        w = min(512, n - a)
        out.append((a, w))
        a += w
    return out


def build(nc, T):
    """Emit the per-core program. T = local tokens (multiple of 512)."""
    TH = T + WIN                      # with halo
    NC_ = T // WIN                    # chunks
    HT = nc.dram_tensor("HT", [DIMS, TH], bf16, kind="ExternalInput")
    WQ = nc.dram_tensor("WQ", [DIMS, DIMS], bf16, kind="ExternalInput")
    WK = nc.dram_tensor("WK", [DIMS, DIMS], bf16, kind="ExternalInput")
    WV = nc.dram_tensor("WV", [DIMS, DIMS], bf16, kind="ExternalInput")
    WO = nc.dram_tensor("WO", [DIMS, DIMS], bf16, kind="ExternalInput")
    COS = nc.dram_tensor("COS", [HD, TH], bf16, kind="ExternalInput")
    SINS = nc.dram_tensor("SINS", [HD, TH], bf16, kind="ExternalInput")
    TRI23 = nc.dram_tensor("TRI23", [128, WIN + 128], bf16,
                           kind="ExternalInput")
    PGATE = nc.dram_tensor("PGATE", [128, 1], f32, kind="ExternalInput")
    ONESM = nc.dram_tensor("ONESM", [128, 128], bf16, kind="ExternalInput")
    OUT = nc.dram_tensor("OUT", [T, DIMS], bf16, kind="ExternalOutput")

    QTS = nc.dram_tensor("QTS", [HEADS, HD, T], bf16)    # raw (pre-RoPE) Q^T
    KTS = nc.dram_tensor("KTS", [HEADS, HD, TH], bf16)   # raw K^T (with halo)
    VS = nc.dram_tensor("VS", [TH, DIMS], bf16)          # V natural

    with tile.TileContext(nc) as tc:
        with tc.tile_pool(name="cst", bufs=1) as cst, \
             tc.tile_pool(name="qk", bufs=2) as qk:
            tri23 = cst.tile([128, WIN + 128], bf16)
            pgate = cst.tile([128, 1], f32)
            onesm = cst.tile([128, 128], bf16)
            cosb = cst.tile([128, 1, TH], bf16)
            sinb = cst.tile([128, 1, TH], bf16)
            nc.gpsimd.dma_start(tri23[:], TRI23[:])
            nc.gpsimd.dma_start(pgate[:], PGATE[:])
            nc.gpsimd.dma_start(onesm[:], ONESM[:])
            nc.gpsimd.dma_start(cosb[:, 0], COS[:])
            nc.gpsimd.dma_start(sinb[:, 0], SINS[:])

            # ---------------- P1: projections ----------------
            with tc.tile_pool(name="p1", bufs=1) as p1, \
                 tc.tile_pool(name="wp", bufs=2) as wp, \
                 tc.tile_pool(name="st", bufs=2) as st, \
                 tc.tile_pool(name="pp", bufs=4, space="PSUM") as pp:
                def load_wt(W_, ob):
                    wt = wp.tile([128, IB, 128], bf16, name="w")
                    nc.sync.dma_start(
                        wt[:], W_[:, ob * 128:(ob + 1) * 128]
                        .rearrange("(ib p) o -> p ib o", p=128))
                    return wt

                # first weight tile ahead of the big hidden-state load so
                # the PE can start as soon as token group 0 lands
                wt_next = load_wt(WK, 0)
                ht = p1.tile([128, IB, TH], bf16)
                for a, w in tok_tiles(TH):
                    nc.sync.dma_start(
                        ht[:, :, a:a + w],
                        HT[:, a:a + w].rearrange("(ib p) t -> p ib t", p=128))

                # K^T / Q^T: lhsT = W column block [128in, 128out].
                # RoPE is applied here, once per head row: rotate-half via
                # SBUF->SBUF partition-swap DMA, then 3 in-place DVE ops
                # (DVE is otherwise idle in P1). pos0 = column into COS/SINS.
                seq = [(WK, KTS, 0, TH, ob) for ob in range(HEADS)] + \
                      [(WQ, QTS, WIN, T, ob) for ob in range(HEADS)]
                for idx, (W_, DST, t0, tlen, ob) in enumerate(seq):
                    wt = wt_next
                    if idx + 1 < len(seq):
                        nw, _, _, _, nob = seq[idx + 1]
                        wt_next = load_wt(nw, nob)
                    stg = st.tile([128, TH], bf16, name="stg")
                    for a, w in tok_tiles(tlen):
                        ps = pp.tile([128, 512], f32, name="pp")
                        for ib in range(IB):
                            nc.tensor.matmul(
                                ps[:, :w], wt[:, ib],
                                ht[:, ib, t0 + a:t0 + a + w],
                                start=(ib == 0), stop=(ib == IB - 1))
                        nc.scalar.copy(stg[:, a:a + w], ps[:, :w])
                    rot = st.tile([128, TH], bf16, name="rot")
                    nc.sync.dma_start(rot[0:64, 0:tlen], stg[64:128, 0:tlen])
                    nc.sync.dma_start(rot[64:128, 0:tlen], stg[0:64, 0:tlen])
                    cs = cosb[:, 0, t0:t0 + tlen]
                    sn = sinb[:, 0, t0:t0 + tlen]
                    nc.vector.tensor_mul(rot[:, 0:tlen], rot[:, 0:tlen], sn)
                    nc.vector.tensor_mul(stg[:, 0:tlen], stg[:, 0:tlen], cs)
                    nc.vector.tensor_add(stg[:, 0:tlen], stg[:, 0:tlen],
                                         rot[:, 0:tlen])
                    nc.sync.dma_start(DST[ob][:, 0:tlen], stg[:, 0:tlen])

                # V natural: lhsT = hT block [128in, 128tok], rhs = Wv rows
                def load_wv(og):
                    wv = wp.tile([128, IB, 512], bf16, name="wv")
                    nc.sync.dma_start(
                        wv[:], WV[:, og * 512:(og + 1) * 512]
                        .rearrange("(ib p) d -> p ib d", p=128))
                    return wv

                wv_next = load_wv(0)
                for og in range(4):
                    wv = wv_next
                    if og + 1 < 4:
                        wv_next = load_wv(og + 1)
                    for tb in range(TH // 128):
                        ps = pp.tile([128, 512], f32, name="pp")
                        for ib in range(IB):
                            nc.tensor.matmul(
                                ps[:], ht[:, ib, tb * 128:(tb + 1) * 128],
                                wv[:, ib, :],
                                start=(ib == 0), stop=(ib == IB - 1))
                        stgv = st.tile([128, 512], bf16, name="stgv")
                        nc.scalar.copy(stgv[:], ps[:])
                        nc.sync.dma_start(
                            VS[tb * 128:(tb + 1) * 128,
                               og * 512:(og + 1) * 512], stgv[:])

            # ---------------- P2 + P3 fused ----------------
            with tc.tile_pool(name="wop", bufs=1) as wop, \
                 tc.tile_pool(name="pbp", bufs=5) as pbp, \
                 tc.tile_pool(name="pad", bufs=2) as padp, \
                 tc.tile_pool(name="pad2", bufs=4) as padp2, \
                 tc.tile_pool(name="ob", bufs=2) as obp, \
                 tc.tile_pool(name="otp", bufs=10) as otp, \
                 tc.tile_pool(name="st3", bufs=2) as st3, \
                 tc.tile_pool(name="ps_s", bufs=2, space="PSUM") as ps_s, \
                 tc.tile_pool(name="ps_po", bufs=1, space="PSUM") as ps_po, \
                 tc.tile_pool(name="ps_p", bufs=2, space="PSUM") as ps_p:

                def kq_issue(SRC, c0, which):
                    t = qk.tile([128, HEADS, WIN], bf16, name=which)
                    nc.gpsimd.dma_start(
                        t[:], SRC[:, :, c0:c0 + WIN]
                        .rearrange("h d w -> d h w"))
                    return t

                def v_issue(w0):
                    v = qk.tile([128, 2, DIMS], bf16, name="v")
                    nc.gpsimd.dma_start(
                        v[:], VS[w0:w0 + WIN].rearrange("(tb p) c -> p tb c",
                                                        p=128))
                    return v

                # prologue: halo window + chunk-0 tiles. K/Q loads first
                # (their scratch is ready mid-P1, so these drain early); the
                # V loads and the big Wo load go behind them in the queue.
                kt_prev = kq_issue(KTS, 0, "kt")
                kt_cur = kq_issue(KTS, WIN, "kt")
                qt = kq_issue(QTS, 0, "qt")
                v_prev = v_issue(0)
                v_cur = v_issue(WIN)
                wo = wop.tile([128, IB, DIMS], bf16)
                for nt in range(4):
                    nc.gpsimd.dma_start(
                        wo[:, :, nt * 512:(nt + 1) * 512],
                        WO[:, nt * 512:(nt + 1) * 512]
                        .rearrange("(ib p) d -> p ib d", p=128))

                W2 = 2 * WIN
                for c in range(NC_):
                    if c + 1 < NC_:
                        kt_next = kq_issue(KTS, WIN + (c + 1) * WIN, "kt")
                        qt_next = kq_issue(QTS, (c + 1) * WIN, "qt")
                        v_next = v_issue(WIN + (c + 1) * WIN)

                    kts = [kt_prev, kt_prev, kt_cur, kt_cur]
                    vs = [v_prev, v_prev, v_cur, v_cur]
                    ots_c = []
                    pend = None  # (h0, [(h, pb, pa2) x2]) awaiting den+O

                    def den_o(pair):
                        pod = ps_po.tile([128, 4, WIN], f32, name="pod")
                        for i, (h, pb, pa2) in enumerate(pair):
                            nc.tensor.matmul(pod[:, 2 + i], onesm[:], pa2[:],
                                             start=True, stop=True)
                            for kb in range(3):
                                nc.tensor.matmul(
                                    pod[:, i],
                                    vs[kb][:, kb % 2, h * 128:(h + 1) * 128],
                                    pb[:, kb * WIN:(kb + 1) * WIN],
                                    start=(kb == 0), stop=(kb == 2))
                            nc.tensor.matmul(
                                pod[:, i, 128:WIN],
                                vs[3][:, 1, h * 128:(h + 1) * 128],
                                pb[:, 3 * WIN:3 * WIN + 128], start=False,
                                stop=True, skip_group_check=True)
                        rb = obp.tile([128, 2, WIN], f32, name="rb")
                        with nc.allow_low_precision("softmax denominator"):
                            nc.vector.reciprocal(rb[:], pod[:, 2:4])
                        ot = otp.tile([128, 2, WIN], bf16, name="ot")
                        nc.vector.tensor_mul(ot[:], pod[:, 0:2], rb[:])
                        ots_c.append(ot)

                    for h0 in range(0, HEADS, 2):
                        pair = []
                        for h in (h0, h0 + 1):
                            # scores packed flat in one 2-bank tile; key
                            # block 3 only sees queries 128:256 (queries
                            # 0:128 are fully causal-masked against keys
                            # 128:256), packed at cols 768:896 so a single
                            # exp and a single mask op cover everything
                            ps = ps_s.tile([128, 4 * WIN], f32, name="ps")
                            for kb in range(3):
                                nc.tensor.matmul(
                                    ps[:, kb * WIN:(kb + 1) * WIN],
                                    kts[kb][:, h,
                                            (kb % 2) * 128:(kb % 2) * 128 + 128],
                                    qt[:, h], start=True, stop=True)
                            nc.tensor.matmul(
                                ps[:, 3 * WIN:3 * WIN + 128],
                                kts[3][:, h, 128:256],
                                qt[:, h, 128:WIN], start=True, stop=True)
                            pb = pbp.tile([128, 4 * WIN], bf16, name="pb")
                            if c == 0:
                                nc.scalar.activation(
                                    pb[:, 0:2 * WIN], ps[:, 0:2 * WIN],
                                    AF.Exp, bias=pgate[:], scale=ISQ)
                                nc.scalar.activation(
                                    pb[:, 2 * WIN:3 * WIN + 128],
                                    ps[:, 2 * WIN:3 * WIN + 128],
                                    AF.Exp, scale=ISQ)
                            else:
                                nc.scalar.activation(
                                    pb[:, 0:3 * WIN + 128],
                                    ps[:, 0:3 * WIN + 128], AF.Exp, scale=ISQ)
                            nc.vector.tensor_mul(
                                pb[:, 2 * WIN:3 * WIN + 128],
                                pb[:, 2 * WIN:3 * WIN + 128], tri23[:])
                            # denominator pre-add on DVE (kb3 live half only)
                            pa = padp.tile([128, WIN], bf16, name="pa")
                            nc.vector.tensor_add(pa[:], pb[:, 0:WIN],
                                                 pb[:, WIN:2 * WIN])
                            pa2 = padp2.tile([128, WIN], bf16, name="pa2")
                            nc.vector.tensor_add(pa2[:], pa[:],
                                                 pb[:, 2 * WIN:3 * WIN])
                            nc.vector.tensor_add(
                                pa2[:, 128:WIN], pa2[:, 128:WIN],
                                pb[:, 3 * WIN:3 * WIN + 128])
                            pair.append((h, pb, pa2))
                        if pend is not None:
                            den_o(pend)
                        pend = pair
                    den_o(pend)

                    # P3: project this chunk's 256 output rows against Wo
                    for tt in range(2):
                        stg = st3.tile([128, DIMS], bf16, name="st3")
                        for nt in range(4):
                            ps = ps_p.tile([128, 512], f32, name="pp3")
                            for h in range(HEADS):
                                nc.tensor.matmul(
                                    ps[:],
                                    ots_c[h // 2][:, h % 2,
                                                  tt * 128:(tt + 1) * 128],
                                    wo[:, h, nt * 512:(nt + 1) * 512],
                                    start=(h == 0), stop=(h == HEADS - 1))
                            nc.scalar.copy(
                                stg[:, nt * 512:(nt + 1) * 512], ps[:])
                        nc.sync.dma_start(
                            OUT[c * WIN + tt * 128:c * WIN + (tt + 1) * 128, :],
                            stg[:])
                    if c + 1 < NC_:
                        kt_prev, v_prev = kt_cur, v_cur
                        kt_cur, v_cur, qt = kt_next, v_next, qt_next
    return nc


def _host_inputs(hidden_states, Wq, Wk, Wv, Wo, T):
    """Build the 8 per-core input maps."""
    TH = T + WIN
    inv_freq = 1.0 / (THETA ** (np.arange(0, HD, 2, dtype=np.float32) / HD))

    qq = np.arange(WIN)[None, :]
    kk = np.arange(128)[:, None]
    # [mask for current-chunk keys 0:128 vs all 256 queries |
    #  mask for keys 128:256 vs the live queries 128:256]
    tri23 = np.concatenate(
        [(qq >= kk), (qq[:, 128:] >= kk + 128)], 1).astype(ml_dtypes.bfloat16)
    onesm_bf = np.ones((128, 128), ml_dtypes.bfloat16)

    Wq, Wk, Wv, Wo = (np.asarray(w, np.float32).astype(ml_dtypes.bfloat16)
                      for w in (Wq, Wk, Wv, Wo))
    in_maps = []
    for core in range(8):
        b, sh = divmod(core, NSH)
        t0 = sh * T
        hs = np.zeros((TH, DIMS), np.float32)
        lo = max(0, t0 - WIN)
        hs[WIN - (t0 - lo):] = hidden_states[b, lo:t0 + T]
        hT = np.ascontiguousarray(hs.T).astype(ml_dtypes.bfloat16)

        pos = np.arange(t0 - WIN, t0 + T, dtype=np.float32)
        f = np.outer(inv_freq, pos)                      # [64, TH]
        cos = np.concatenate([np.cos(f), np.cos(f)], 0)  # [128, TH]
        sin = np.sin(f)
        sins = np.concatenate([-sin, sin], 0)
        pg = np.full((128, 1), -1e30 if sh == 0 else 0.0, np.float32)
        in_maps.append({
            "HT": hT, "WQ": Wq, "WK": Wk, "WV": Wv, "WO": Wo,
            "COS": cos.astype(ml_dtypes.bfloat16),
            "SINS": sins.astype(ml_dtypes.bfloat16),
            "TRI23": tri23, "PGATE": pg, "ONESM": onesm_bf,
        })
    return in_maps


_CACHE = {}


def run(hidden_states, Wq, Wk, Wv, Wo, T=S // NSH, **spmd_kwargs):
    key = T
    if key not in _CACHE:
        nc = bacc.Bacc(None)
        build(nc, T)
        nc.finalize()
        _CACHE[key] = nc
    nc = _CACHE[key]
    in_maps = _host_inputs(hidden_states, Wq, Wk, Wv, Wo, T)
    res = run_bass_kernel_spmd(nc, in_maps, core_ids=list(range(8)), **spmd_kwargs)
    outs = [res.results[i]["OUT"] for i in range(8)]
    full = np.empty((B, NSH * T, DIMS), np.float32)
    for core in range(8):
        b, sh = divmod(core, NSH)
        full[b, sh * T:(sh + 1) * T] = np.asarray(outs[core]).astype(np.float32)
    return full, res


def kernel(hidden_states, Wq, Wk, Wv, Wo):
    out, _ = run(np.asarray(hidden_states), Wq, Wk, Wv, Wo)
    return out


# revision 42
# speedup vs baseline: 1.2812x; 1.0051x over previous
"""Block sliding-window attention on 8 TRN2 NeuronCores.

Sharding: sequence-parallel. 8 shards = (batch b in {0,1}) x (quarter s in
0..3); each core owns 2048 consecutive tokens of one batch plus a 256-token
K/V halo from the previous quarter (zeros + -inf gate for the first quarter).
No collectives: each core computes its tokens' full output rows.

Per-core pipeline (all matmuls bf16: full PE rate):
  P1: K^T/Q^T = W^T @ hiddenT (head-transposed layout, raw), V = hidden @ Wv
      (natural layout), staged through DRAM scratch. hiddenT is streamed in
      5 token-group DMAs so the first matmuls start ~7us in; per head-column
      the 16 weight tiles arrive as one DMA and stay resident.
  P2+P3 fused per 256-token chunk: RoPE on Q/K (rot-half via partition-offset
      DMA reload + pre-signed sin, cos/sin broadcast via stride-0 APs), then
      per head: S^T = K Q^T per 128-key block into one 2-bank PSUM tile,
      single exp on ACT (scale=1/sqrt(128), -1e30 bias gates the no-previous
      case), 0/1 triangular mask multiply on DVE, denominator via DVE
      pre-add + one all-ones matmul (broadcasts across partitions),
      O^T = V^T P^T, normalize with DVE reciprocal; afterwards the chunk's
      256 output rows are projected against SBUF-resident Wo and stored.
      P2 loads ride the Pool queue; stores ride the sync queue.
"""
import sys

try:
    import concourse  # noqa: F401
except ImportError:
    sys.path.insert(0, '/opt/trn_rl_repo')

import ml_dtypes
import numpy as np

import concourse.bacc as bacc
import concourse.mybir as mybir
import concourse.tile as tile
from concourse.bass_utils import run_bass_kernel_spmd

f32 = mybir.dt.float32
AF = mybir.ActivationFunctionType
bf16 = mybir.dt.bfloat16

DIMS = 2048
HEADS = 16
HD = 128           # head dim
WIN = 256          # window / chunk
B, S = 2, 8192
NSH = 4            # seq shards per batch
THETA = 10000.0
ISQ = float(1.0 / np.sqrt(HD))
IB = DIMS // 128   # 16 input-dim blocks


def tok_tiles(n):
    out, a = [], 0
    while a < n:
        w = min(512, n - a)
        out.append((a, w))
        a += w
    return out


def build(nc, T):
    """Emit the per-core program. T = local tokens (multiple of 512)."""
    TH = T + WIN                      # with halo
    NC_ = T // WIN                    # chunks
    HT = nc.dram_tensor("HT", [DIMS, TH], bf16, kind="ExternalInput")
    WQ = nc.dram_tensor("WQ", [DIMS, DIMS], bf16, kind="ExternalInput")
    WK = nc.dram_tensor("WK", [DIMS, DIMS], bf16, kind="ExternalInput")
    WV = nc.dram_tensor("WV", [DIMS, DIMS], bf16, kind="ExternalInput")
    WO = nc.dram_tensor("WO", [DIMS, DIMS], bf16, kind="ExternalInput")
    COS = nc.dram_tensor("COS", [HD, TH], bf16, kind="ExternalInput")
    SINS = nc.dram_tensor("SINS", [HD, TH], bf16, kind="ExternalInput")
    TRI23 = nc.dram_tensor("TRI23", [128, WIN + 128], bf16,
                           kind="ExternalInput")
    PGATE = nc.dram_tensor("PGATE", [128, 1], f32, kind="ExternalInput")
    ONESM = nc.dram_tensor("ONESM", [128, 128], bf16, kind="ExternalInput")
    OUT = nc.dram_tensor("OUT", [T, DIMS], bf16, kind="ExternalOutput")

    QTS = nc.dram_tensor("QTS", [HEADS, HD, T], bf16)    # raw (pre-RoPE) Q^T
    KTS = nc.dram_tensor("KTS", [HEADS, HD, TH], bf16)   # raw K^T (with halo)
    VS = nc.dram_tensor("VS", [TH, DIMS], bf16)          # V natural

    with tile.TileContext(nc) as tc:
        with tc.tile_pool(name="cst", bufs=1) as cst, \
             tc.tile_pool(name="qk", bufs=2) as qk:
            tri23 = cst.tile([128, WIN + 128], bf16)
            pgate = cst.tile([128, 1], f32)
            onesm = cst.tile([128, 128], bf16)
            cosb = cst.tile([128, 1, TH], bf16)
            sinb = cst.tile([128, 1, TH], bf16)
            nc.gpsimd.dma_start(tri23[:], TRI23[:])
            nc.gpsimd.dma_start(pgate[:], PGATE[:])
            nc.gpsimd.dma_start(onesm[:], ONESM[:])
            nc.gpsimd.dma_start(cosb[:, 0], COS[:])
            nc.gpsimd.dma_start(sinb[:, 0], SINS[:])

            # ---------------- P1: projections ----------------
            with tc.tile_pool(name="p1", bufs=1) as p1, \
                 tc.tile_pool(name="wp", bufs=2) as wp, \
                 tc.tile_pool(name="st", bufs=2) as st, \
                 tc.tile_pool(name="pp", bufs=4, space="PSUM") as pp:
                def load_wt(W_, ob):
                    wt = wp.tile([128, IB, 128], bf16, name="w")
                    nc.sync.dma_start(
                        wt[:], W_[:, ob * 128:(ob + 1) * 128]
                        .rearrange("(ib p) o -> p ib o", p=128))
                    return wt

                # first weight tile ahead of the big hidden-state load so
                # the PE can start as soon as token group 0 lands
                wt_next = load_wt(WK, 0)
                ht = p1.tile([128, IB, TH], bf16)
                for a, w in [(0, 256), (256, 256)] + tok_tiles(TH)[1:]:
                    nc.sync.dma_start(
                        ht[:, :, a:a + w],
                        HT[:, a:a + w].rearrange("(ib p) t -> p ib t", p=128))

                # K^T / Q^T: lhsT = W column block [128in, 128out].
                # RoPE is applied here, once per head row: rotate-half via
                # SBUF->SBUF partition-swap DMA, then 3 in-place DVE ops
                # (DVE is otherwise idle in P1). pos0 = column into COS/SINS.
                seq = [(WK, KTS, 0, TH, ob) for ob in range(HEADS)] + \
                      [(WQ, QTS, WIN, T, ob) for ob in range(HEADS)]
                for idx, (W_, DST, t0, tlen, ob) in enumerate(seq):
                    wt = wt_next
                    if idx + 1 < len(seq):
                        nw, _, _, _, nob = seq[idx + 1]
                        wt_next = load_wt(nw, nob)
                    stg = st.tile([128, TH], bf16, name="stg")
                    for a, w in tok_tiles(tlen):
                        ps = pp.tile([128, 512], f32, name="pp")
                        for ib in range(IB):
                            nc.tensor.matmul(
                                ps[:, :w], wt[:, ib],
                                ht[:, ib, t0 + a:t0 + a + w],
                                start=(ib == 0), stop=(ib == IB - 1))
                        nc.scalar.copy(stg[:, a:a + w], ps[:, :w])
                    rot = st.tile([128, TH], bf16, name="rot", bufs=1)
                    nc.sync.dma_start(rot[0:64, 0:tlen], stg[64:128, 0:tlen])
                    nc.sync.dma_start(rot[64:128, 0:tlen], stg[0:64, 0:tlen])
                    cs = cosb[:, 0, t0:t0 + tlen]
                    sn = sinb[:, 0, t0:t0 + tlen]
                    nc.vector.tensor_mul(rot[:, 0:tlen], rot[:, 0:tlen], sn)
                    nc.vector.tensor_mul(stg[:, 0:tlen], stg[:, 0:tlen], cs)
                    nc.vector.tensor_add(stg[:, 0:tlen], stg[:, 0:tlen],
                                         rot[:, 0:tlen])
                    nc.sync.dma_start(DST[ob][:, 0:tlen], stg[:, 0:tlen])

                # V natural: lhsT = hT block [128in, 128tok], rhs = Wv rows
                def load_wv(og):
                    wv = wp.tile([128, IB, 512], bf16, name="wv")
                    nc.sync.dma_start(
                        wv[:], WV[:, og * 512:(og + 1) * 512]
                        .rearrange("(ib p) d -> p ib d", p=128))
                    return wv

                wv_next = load_wv(0)
                for og in range(4):
                    wv = wv_next
                    if og + 1 < 4:
                        wv_next = load_wv(og + 1)
                    for tb in range(TH // 128):
                        ps = pp.tile([128, 512], f32, name="pp")
                        for ib in range(IB):
                            nc.tensor.matmul(
                                ps[:], ht[:, ib, tb * 128:(tb + 1) * 128],
                                wv[:, ib, :],
                                start=(ib == 0), stop=(ib == IB - 1))
                        stgv = st.tile([128, 512], bf16, name="stgv")
                        nc.scalar.copy(stgv[:], ps[:])
                        nc.sync.dma_start(
                            VS[tb * 128:(tb + 1) * 128,
                               og * 512:(og + 1) * 512], stgv[:])

            # ---------------- P2 + P3 fused ----------------
            with tc.tile_pool(name="wop", bufs=1) as wop, \
                 tc.tile_pool(name="pbp", bufs=4) as pbp, \
                 tc.tile_pool(name="pad", bufs=2) as padp, \
                 tc.tile_pool(name="pad2", bufs=4) as padp2, \
                 tc.tile_pool(name="ob", bufs=2) as obp, \
                 tc.tile_pool(name="otp", bufs=10) as otp, \
                 tc.tile_pool(name="st3", bufs=2) as st3, \
                 tc.tile_pool(name="ps_s", bufs=2, space="PSUM") as ps_s, \
                 tc.tile_pool(name="ps_po", bufs=1, space="PSUM") as ps_po, \
                 tc.tile_pool(name="ps_p", bufs=2, space="PSUM") as ps_p:

                def kq_issue(SRC, c0, which):
                    t = qk.tile([128, HEADS, WIN], bf16, name=which,
                                bufs=3 if which == "kt" else 2)
                    nc.gpsimd.dma_start(
                        t[:], SRC[:, :, c0:c0 + WIN]
                        .rearrange("h d w -> d h w"))
                    return t

                def v_issue(w0):
                    v = qk.tile([128, 2, DIMS], bf16, name="v", bufs=3)
                    nc.gpsimd.dma_start(
                        v[:], VS[w0:w0 + WIN].rearrange("(tb p) c -> p tb c",
                                                        p=128))
                    return v

                # prologue: halo window + chunk-0 tiles. K/Q loads first
                # (their scratch is ready mid-P1, so these drain early); the
                # V loads and the big Wo load go behind them in the queue.
                kt_prev = kq_issue(KTS, 0, "kt")
                kt_cur = kq_issue(KTS, WIN, "kt")
                qt = kq_issue(QTS, 0, "qt")
                v_prev = v_issue(0)
                v_cur = v_issue(WIN)
                wo = wop.tile([128, IB, DIMS], bf16)
                for nt in range(4):
                    nc.gpsimd.dma_start(
                        wo[:, :, nt * 512:(nt + 1) * 512],
                        WO[:, nt * 512:(nt + 1) * 512]
                        .rearrange("(ib p) d -> p ib d", p=128))

                W2 = 2 * WIN
                for c in range(NC_):
                    if c + 1 < NC_:
                        kt_next = kq_issue(KTS, WIN + (c + 1) * WIN, "kt")
                        qt_next = kq_issue(QTS, (c + 1) * WIN, "qt")
                        v_next = v_issue(WIN + (c + 1) * WIN)

                    kts = [kt_prev, kt_prev, kt_cur, kt_cur]
                    vs = [v_prev, v_prev, v_cur, v_cur]
                    ots_c = []
                    pend = None  # (h0, [(h, pb, pa2) x2]) awaiting den+O

                    def den_o(pair):
                        pod = ps_po.tile([128, 4, WIN], f32, name="pod")
                        for i, (h, pb, pa2) in enumerate(pair):
                            nc.tensor.matmul(pod[:, 2 + i], onesm[:], pa2[:],
                                             start=True, stop=True)
                            for kb in range(3):
                                nc.tensor.matmul(
                                    pod[:, i],
                                    vs[kb][:, kb % 2, h * 128:(h + 1) * 128],
                                    pb[:, kb * WIN:(kb + 1) * WIN],
                                    start=(kb == 0), stop=(kb == 2))
                            nc.tensor.matmul(
                                pod[:, i, 128:WIN],
                                vs[3][:, 1, h * 128:(h + 1) * 128],
                                pb[:, 3 * WIN:3 * WIN + 128], start=False,
                                stop=True, skip_group_check=True)
                        rb = obp.tile([128, 2, WIN], f32, name="rb")
                        nc.vector.reciprocal_approx_fast(rb[:], pod[:, 2:4])
                        ot = otp.tile([128, 2, WIN], bf16, name="ot")
                        nc.vector.tensor_mul(ot[:], pod[:, 0:2], rb[:])
                        ots_c.append(ot)

                    for h0 in range(0, HEADS, 2):
                        pair = []
                        for h in (h0, h0 + 1):
                            # scores packed flat in one 2-bank tile; key
                            # block 3 only sees queries 128:256 (queries
                            # 0:128 are fully causal-masked against keys
                            # 128:256), packed at cols 768:896 so a single
                            # exp and a single mask op cover everything
                            ps = ps_s.tile([128, 4 * WIN], f32, name="ps")
                            for kb in range(3):
                                nc.tensor.matmul(
                                    ps[:, kb * WIN:(kb + 1) * WIN],
                                    kts[kb][:, h,
                                            (kb % 2) * 128:(kb % 2) * 128 + 128],
                                    qt[:, h], start=True, stop=True)
                            nc.tensor.matmul(
                                ps[:, 3 * WIN:3 * WIN + 128],
                                kts[3][:, h, 128:256],
                                qt[:, h, 128:WIN], start=True, stop=True)
                            pb = pbp.tile([128, 4 * WIN], bf16, name="pb")
                            if c == 0:
                                nc.scalar.activation(
                                    pb[:, 0:2 * WIN], ps[:, 0:2 * WIN],
                                    AF.Exp, bias=pgate[:], scale=ISQ)
                                nc.scalar.activation(
                                    pb[:, 2 * WIN:3 * WIN + 128],
                                    ps[:, 2 * WIN:3 * WIN + 128],
                                    AF.Exp, scale=ISQ)
                            else:
                                nc.scalar.activation(
                                    pb[:, 0:3 * WIN + 128],
                                    ps[:, 0:3 * WIN + 128], AF.Exp, scale=ISQ)
                            nc.vector.tensor_mul(
                                pb[:, 2 * WIN:3 * WIN + 128],
                                pb[:, 2 * WIN:3 * WIN + 128], tri23[:])
                            # denominator pre-add on DVE (kb3 live half only)
                            pa = padp.tile([128, WIN], bf16, name="pa")
                            nc.vector.tensor_add(pa[:], pb[:, 0:WIN],
                                                 pb[:, WIN:2 * WIN])
                            pa2 = padp2.tile([128, WIN], bf16, name="pa2")
                            nc.vector.tensor_add(pa2[:], pa[:],
                                                 pb[:, 2 * WIN:3 * WIN])
                            nc.vector.tensor_add(
                                pa2[:, 128:WIN], pa2[:, 128:WIN],
                                pb[:, 3 * WIN:3 * WIN + 128])
                            pair.append((h, pb, pa2))
                        if pend is not None:
                            den_o(pend)
                        pend = pair
                    den_o(pend)

                    # P3: project this chunk's 256 output rows against Wo
                    for tt in range(2):
                        stg = st3.tile([128, DIMS], bf16, name="st3")
                        for nt in range(4):
                            ps = ps_p.tile([128, 512], f32, name="pp3")
                            for h in range(HEADS):
                                nc.tensor.matmul(
                                    ps[:],
                                    ots_c[h // 2][:, h % 2,
                                                  tt * 128:(tt + 1) * 128],
                                    wo[:, h, nt * 512:(nt + 1) * 512],
                                    start=(h == 0), stop=(h == HEADS - 1))
                            nc.vector.tensor_copy(
                                stg[:, nt * 512:(nt + 1) * 512], ps[:])
                        r0 = c * WIN + tt * 128
                        if c == NC_ - 1 and tt == 1:
                            # split the final store so the tail drains sooner
                            nc.sync.dma_start(OUT[r0:r0 + 128, 0:1024],
                                              stg[:, 0:1024])
                            nc.sync.dma_start(OUT[r0:r0 + 128, 1024:2048],
                                              stg[:, 1024:2048])
                        else:
                            nc.sync.dma_start(OUT[r0:r0 + 128, :], stg[:])
                    if c + 1 < NC_:
                        kt_prev, v_prev = kt_cur, v_cur
                        kt_cur, v_cur, qt = kt_next, v_next, qt_next
    return nc


def _host_inputs(hidden_states, Wq, Wk, Wv, Wo, T):
    """Build the 8 per-core input maps."""
    TH = T + WIN
    inv_freq = 1.0 / (THETA ** (np.arange(0, HD, 2, dtype=np.float32) / HD))

    qq = np.arange(WIN)[None, :]
    kk = np.arange(128)[:, None]
    # [mask for current-chunk keys 0:128 vs all 256 queries |
    #  mask for keys 128:256 vs the live queries 128:256]
    tri23 = np.concatenate(
        [(qq >= kk), (qq[:, 128:] >= kk + 128)], 1).astype(ml_dtypes.bfloat16)
    onesm_bf = np.ones((128, 128), ml_dtypes.bfloat16)

    Wq, Wk, Wv, Wo = (np.asarray(w, np.float32).astype(ml_dtypes.bfloat16)
                      for w in (Wq, Wk, Wv, Wo))
    in_maps = []
    for core in range(8):
        b, sh = divmod(core, NSH)
        t0 = sh * T
        hs = np.zeros((TH, DIMS), np.float32)
        lo = max(0, t0 - WIN)
        hs[WIN - (t0 - lo):] = hidden_states[b, lo:t0 + T]
        hT = np.ascontiguousarray(hs.T).astype(ml_dtypes.bfloat16)

        pos = np.arange(t0 - WIN, t0 + T, dtype=np.float32)
        f = np.outer(inv_freq, pos)                      # [64, TH]
        cos = np.concatenate([np.cos(f), np.cos(f)], 0)  # [128, TH]
        sin = np.sin(f)
        sins = np.concatenate([-sin, sin], 0)
        pg = np.full((128, 1), -1e30 if sh == 0 else 0.0, np.float32)
        in_maps.append({
            "HT": hT, "WQ": Wq, "WK": Wk, "WV": Wv, "WO": Wo,
            "COS": cos.astype(ml_dtypes.bfloat16),
            "SINS": sins.astype(ml_dtypes.bfloat16),
            "TRI23": tri23, "PGATE": pg, "ONESM": onesm_bf,
        })
    return in_maps


_CACHE = {}


def run(hidden_states, Wq, Wk, Wv, Wo, T=S // NSH, **spmd_kwargs):
    key = T
    if key not in _CACHE:
        nc = bacc.Bacc(None)
        build(nc, T)
        nc.finalize()
        _CACHE[key] = nc
    nc = _CACHE[key]
    in_maps = _host_inputs(hidden_states, Wq, Wk, Wv, Wo, T)
    res = run_bass_kernel_spmd(nc, in_maps, core_ids=list(range(8)), **spmd_kwargs)
    outs = [res.results[i]["OUT"] for i in range(8)]
    full = np.empty((B, NSH * T, DIMS), np.float32)
    for core in range(8):
        b, sh = divmod(core, NSH)
        full[b, sh * T:(sh + 1) * T] = np.asarray(outs[core]).astype(np.float32)
    return full, res


def kernel(hidden_states, Wq, Wk, Wv, Wo):
    out, _ = run(np.asarray(hidden_states), Wq, Wk, Wv, Wo)
    return out


# revision 45
# speedup vs baseline: 1.2829x; 1.0013x over previous
"""Block sliding-window attention on 8 TRN2 NeuronCores.

Sharding: sequence-parallel. 8 shards = (batch b in {0,1}) x (quarter s in
0..3); each core owns 2048 consecutive tokens of one batch plus a 256-token
K/V halo from the previous quarter (zeros + -inf gate for the first quarter).
No collectives: each core computes its tokens' full output rows.

Per-core pipeline (all matmuls bf16: full PE rate):
  P1: K^T/Q^T = W^T @ hiddenT (head-transposed layout, raw), V = hidden @ Wv
      (natural layout), staged through DRAM scratch. hiddenT is streamed in
      5 token-group DMAs so the first matmuls start ~7us in; per head-column
      the 16 weight tiles arrive as one DMA and stay resident.
  P2+P3 fused per 256-token chunk: RoPE on Q/K (rot-half via partition-offset
      DMA reload + pre-signed sin, cos/sin broadcast via stride-0 APs), then
      per head: S^T = K Q^T per 128-key block into one 2-bank PSUM tile,
      single exp on ACT (scale=1/sqrt(128), -1e30 bias gates the no-previous
      case), 0/1 triangular mask multiply on DVE, denominator via DVE
      pre-add + one all-ones matmul (broadcasts across partitions),
      O^T = V^T P^T, normalize with DVE reciprocal; afterwards the chunk's
      256 output rows are projected against SBUF-resident Wo and stored.
      P2 loads ride the Pool queue; stores ride the sync queue.
"""
import sys

try:
    import concourse  # noqa: F401
except ImportError:
    sys.path.insert(0, '/opt/trn_rl_repo')

import ml_dtypes
import numpy as np

import concourse.bacc as bacc
import concourse.mybir as mybir
import concourse.tile as tile
from concourse.bass_utils import run_bass_kernel_spmd

f32 = mybir.dt.float32
AF = mybir.ActivationFunctionType
bf16 = mybir.dt.bfloat16

DIMS = 2048
HEADS = 16
HD = 128           # head dim
WIN = 256          # window / chunk
B, S = 2, 8192
NSH = 4            # seq shards per batch
THETA = 10000.0
ISQ = float(1.0 / np.sqrt(HD))
IB = DIMS // 128   # 16 input-dim blocks


def tok_tiles(n):
    out, a = [], 0
    while a < n:
        w = min(512, n - a)
        out.append((a, w))
        a += w
    return out


def build(nc, T):
    """Emit the per-core program. T = local tokens (multiple of 512)."""
    TH = T + WIN                      # with halo
    NC_ = T // WIN                    # chunks
    HT = nc.dram_tensor("HT", [DIMS, TH], bf16, kind="ExternalInput")
    # WQ/WK pre-tiled on host to [ob, p, ib, o] so each head-column's
    # 16 weight tiles arrive as one fully-contiguous DMA
    WQ = nc.dram_tensor("WQ", [HEADS, 128, IB, 128], bf16,
                        kind="ExternalInput")
    WK = nc.dram_tensor("WK", [HEADS, 128, IB, 128], bf16,
                        kind="ExternalInput")
    WV = nc.dram_tensor("WV", [DIMS, DIMS], bf16, kind="ExternalInput")
    WO = nc.dram_tensor("WO", [DIMS, DIMS], bf16, kind="ExternalInput")
    COS = nc.dram_tensor("COS", [HD, TH], bf16, kind="ExternalInput")
    SINS = nc.dram_tensor("SINS", [HD, TH], bf16, kind="ExternalInput")
    TRI23 = nc.dram_tensor("TRI23", [128, WIN + 128], bf16,
                           kind="ExternalInput")
    PGATE = nc.dram_tensor("PGATE", [128, 1], f32, kind="ExternalInput")
    ONESM = nc.dram_tensor("ONESM", [128, 128], bf16, kind="ExternalInput")
    OUT = nc.dram_tensor("OUT", [T, DIMS], bf16, kind="ExternalOutput")

    QTS = nc.dram_tensor("QTS", [HEADS, HD, T], bf16)    # raw (pre-RoPE) Q^T
    KTS = nc.dram_tensor("KTS", [HEADS, HD, TH], bf16)   # raw K^T (with halo)
    VS = nc.dram_tensor("VS", [TH, DIMS], bf16)          # V natural

    with tile.TileContext(nc) as tc:
        with tc.tile_pool(name="cst", bufs=1) as cst, \
             tc.tile_pool(name="qk", bufs=2) as qk:
            tri23 = cst.tile([128, WIN + 128], bf16)
            pgate = cst.tile([128, 1], f32)
            onesm = cst.tile([128, 128], bf16)
            cosb = cst.tile([128, 1, TH], bf16)
            sinb = cst.tile([128, 1, TH], bf16)
            nc.gpsimd.dma_start(tri23[:], TRI23[:])
            nc.gpsimd.dma_start(pgate[:], PGATE[:])
            nc.gpsimd.dma_start(onesm[:], ONESM[:])
            nc.gpsimd.dma_start(cosb[:, 0], COS[:])
            nc.gpsimd.dma_start(sinb[:, 0], SINS[:])

            # ---------------- P1: projections ----------------
            with tc.tile_pool(name="p1", bufs=1) as p1, \
                 tc.tile_pool(name="wp", bufs=2) as wp, \
                 tc.tile_pool(name="st", bufs=2) as st, \
                 tc.tile_pool(name="pp", bufs=4, space="PSUM") as pp:
                def load_wt(W_, ob):
                    wt = wp.tile([128, IB, 128], bf16, name="w")
                    nc.sync.dma_start(wt[:], W_[ob])
                    return wt

                # first weight tile ahead of the big hidden-state load so
                # the PE can start as soon as token group 0 lands
                wt_next = load_wt(WK, 0)
                ht = p1.tile([128, IB, TH], bf16)
                for a, w in [(0, 256), (256, 256)] + tok_tiles(TH)[1:]:
                    nc.sync.dma_start(
                        ht[:, :, a:a + w],
                        HT[:, a:a + w].rearrange("(ib p) t -> p ib t", p=128))

                # K^T / Q^T: lhsT = W column block [128in, 128out].
                # RoPE is applied here, once per head row: rotate-half via
                # SBUF->SBUF partition-swap DMA, then 3 in-place DVE ops
                # (DVE is otherwise idle in P1). pos0 = column into COS/SINS.
                seq = [(WK, KTS, 0, TH, ob) for ob in range(HEADS)] + \
                      [(WQ, QTS, WIN, T, ob) for ob in range(HEADS)]
                for idx, (W_, DST, t0, tlen, ob) in enumerate(seq):
                    wt = wt_next
                    if idx + 1 < len(seq):
                        nw, _, _, _, nob = seq[idx + 1]
                        wt_next = load_wt(nw, nob)
                    stg = st.tile([128, TH], bf16, name="stg")
                    for a, w in tok_tiles(tlen):
                        ps = pp.tile([128, 512], f32, name="pp")
                        for ib in range(IB):
                            nc.tensor.matmul(
                                ps[:, :w], wt[:, ib],
                                ht[:, ib, t0 + a:t0 + a + w],
                                start=(ib == 0), stop=(ib == IB - 1))
                        nc.scalar.copy(stg[:, a:a + w], ps[:, :w])
                    rot = st.tile([128, TH], bf16, name="rot", bufs=1)
                    nc.sync.dma_start(rot[0:64, 0:tlen], stg[64:128, 0:tlen])
                    nc.sync.dma_start(rot[64:128, 0:tlen], stg[0:64, 0:tlen])
                    cs = cosb[:, 0, t0:t0 + tlen]
                    sn = sinb[:, 0, t0:t0 + tlen]
                    nc.vector.tensor_mul(rot[:, 0:tlen], rot[:, 0:tlen], sn)
                    nc.vector.tensor_mul(stg[:, 0:tlen], stg[:, 0:tlen], cs)
                    nc.vector.tensor_add(stg[:, 0:tlen], stg[:, 0:tlen],
                                         rot[:, 0:tlen])
                    nc.sync.dma_start(DST[ob][:, 0:tlen], stg[:, 0:tlen])

                # V natural: lhsT = hT block [128in, 128tok], rhs = Wv rows
                def load_wv(og):
                    wv = wp.tile([128, IB, 512], bf16, name="wv")
                    nc.sync.dma_start(
                        wv[:], WV[:, og * 512:(og + 1) * 512]
                        .rearrange("(ib p) d -> p ib d", p=128))
                    return wv

                wv_next = load_wv(0)
                for og in range(4):
                    wv = wv_next
                    if og + 1 < 4:
                        wv_next = load_wv(og + 1)
                    for tb in range(TH // 128):
                        ps = pp.tile([128, 512], f32, name="pp")
                        for ib in range(IB):
                            nc.tensor.matmul(
                                ps[:], ht[:, ib, tb * 128:(tb + 1) * 128],
                                wv[:, ib, :],
                                start=(ib == 0), stop=(ib == IB - 1))
                        stgv = st.tile([128, 512], bf16, name="stgv")
                        nc.scalar.copy(stgv[:], ps[:])
                        nc.sync.dma_start(
                            VS[tb * 128:(tb + 1) * 128,
                               og * 512:(og + 1) * 512], stgv[:])

            # ---------------- P2 + P3 fused ----------------
            with tc.tile_pool(name="wop", bufs=1) as wop, \
                 tc.tile_pool(name="pbp", bufs=4) as pbp, \
                 tc.tile_pool(name="pad", bufs=2) as padp, \
                 tc.tile_pool(name="pad2", bufs=4) as padp2, \
                 tc.tile_pool(name="ob", bufs=2) as obp, \
                 tc.tile_pool(name="otp", bufs=10) as otp, \
                 tc.tile_pool(name="st3", bufs=2) as st3, \
                 tc.tile_pool(name="ps_s", bufs=2, space="PSUM") as ps_s, \
                 tc.tile_pool(name="ps_po", bufs=1, space="PSUM") as ps_po, \
                 tc.tile_pool(name="ps_p", bufs=2, space="PSUM") as ps_p:

                def kq_issue(SRC, c0, which):
                    t = qk.tile([128, HEADS, WIN], bf16, name=which,
                                bufs=3 if which == "kt" else 2)
                    nc.gpsimd.dma_start(
                        t[:], SRC[:, :, c0:c0 + WIN]
                        .rearrange("h d w -> d h w"))
                    return t

                def v_issue(w0):
                    v = qk.tile([128, 2, DIMS], bf16, name="v", bufs=3)
                    nc.gpsimd.dma_start(
                        v[:], VS[w0:w0 + WIN].rearrange("(tb p) c -> p tb c",
                                                        p=128))
                    return v

                # prologue: halo window + chunk-0 tiles. K/Q loads first
                # (their scratch is ready mid-P1, so these drain early); the
                # V loads and the big Wo load go behind them in the queue.
                kt_prev = kq_issue(KTS, 0, "kt")
                kt_cur = kq_issue(KTS, WIN, "kt")
                qt = kq_issue(QTS, 0, "qt")
                v_prev = v_issue(0)
                v_cur = v_issue(WIN)
                wo = wop.tile([128, IB, DIMS], bf16)
                for nt in range(4):
                    nc.gpsimd.dma_start(
                        wo[:, :, nt * 512:(nt + 1) * 512],
                        WO[:, nt * 512:(nt + 1) * 512]
                        .rearrange("(ib p) d -> p ib d", p=128))

                W2 = 2 * WIN
                for c in range(NC_):
                    if c + 1 < NC_:
                        kt_next = kq_issue(KTS, WIN + (c + 1) * WIN, "kt")
                        qt_next = kq_issue(QTS, (c + 1) * WIN, "qt")
                        v_next = v_issue(WIN + (c + 1) * WIN)

                    kts = [kt_prev, kt_prev, kt_cur, kt_cur]
                    vs = [v_prev, v_prev, v_cur, v_cur]
                    ots_c = []
                    pend = None  # (h0, [(h, pb, pa2) x2]) awaiting den+O

                    def den_o(pair):
                        pod = ps_po.tile([128, 4, WIN], f32, name="pod")
                        for i, (h, pb, pa2) in enumerate(pair):
                            nc.tensor.matmul(pod[:, 2 + i], onesm[:], pa2[:],
                                             start=True, stop=True)
                            for kb in range(3):
                                nc.tensor.matmul(
                                    pod[:, i],
                                    vs[kb][:, kb % 2, h * 128:(h + 1) * 128],
                                    pb[:, kb * WIN:(kb + 1) * WIN],
                                    start=(kb == 0), stop=(kb == 2))
                            nc.tensor.matmul(
                                pod[:, i, 128:WIN],
                                vs[3][:, 1, h * 128:(h + 1) * 128],
                                pb[:, 3 * WIN:3 * WIN + 128], start=False,
                                stop=True, skip_group_check=True)
                        rb = obp.tile([128, 2, WIN], f32, name="rb")
                        nc.vector.reciprocal_approx_fast(rb[:], pod[:, 2:4])
                        ot = otp.tile([128, 2, WIN], bf16, name="ot")
                        nc.vector.tensor_mul(ot[:], pod[:, 0:2], rb[:])
                        ots_c.append(ot)

                    for h0 in range(0, HEADS, 2):
                        pair = []
                        for h in (h0, h0 + 1):
                            # scores packed flat in one 2-bank tile; key
                            # block 3 only sees queries 128:256 (queries
                            # 0:128 are fully causal-masked against keys
                            # 128:256), packed at cols 768:896 so a single
                            # exp and a single mask op cover everything
                            ps = ps_s.tile([128, 4 * WIN], f32, name="ps")
                            for kb in range(3):
                                nc.tensor.matmul(
                                    ps[:, kb * WIN:(kb + 1) * WIN],
                                    kts[kb][:, h,
                                            (kb % 2) * 128:(kb % 2) * 128 + 128],
                                    qt[:, h], start=True, stop=True)
                            nc.tensor.matmul(
                                ps[:, 3 * WIN:3 * WIN + 128],
                                kts[3][:, h, 128:256],
                                qt[:, h, 128:WIN], start=True, stop=True)
                            pb = pbp.tile([128, 4 * WIN], bf16, name="pb")
                            if c == 0:
                                nc.scalar.activation(
                                    pb[:, 0:2 * WIN], ps[:, 0:2 * WIN],
                                    AF.Exp, bias=pgate[:], scale=ISQ)
                                nc.scalar.activation(
                                    pb[:, 2 * WIN:3 * WIN + 128],
                                    ps[:, 2 * WIN:3 * WIN + 128],
                                    AF.Exp, scale=ISQ)
                            else:
                                nc.scalar.activation(
                                    pb[:, 0:3 * WIN + 128],
                                    ps[:, 0:3 * WIN + 128], AF.Exp, scale=ISQ)
                            nc.vector.tensor_mul(
                                pb[:, 2 * WIN:3 * WIN + 128],
                                pb[:, 2 * WIN:3 * WIN + 128], tri23[:])
                            # denominator pre-add on DVE (kb3 live half only)
                            pa = padp.tile([128, WIN], bf16, name="pa")
                            nc.vector.tensor_add(pa[:], pb[:, 0:WIN],
                                                 pb[:, WIN:2 * WIN])
                            pa2 = padp2.tile([128, WIN], bf16, name="pa2")
                            nc.vector.tensor_add(pa2[:], pa[:],
                                                 pb[:, 2 * WIN:3 * WIN])
                            nc.vector.tensor_add(
                                pa2[:, 128:WIN], pa2[:, 128:WIN],
                                pb[:, 3 * WIN:3 * WIN + 128])
                            pair.append((h, pb, pa2))
                        if pend is not None:
                            den_o(pend)
                        pend = pair
                    den_o(pend)

                    # P3: project this chunk's 256 output rows against Wo
                    for tt in range(2):
                        stg = st3.tile([128, DIMS], bf16, name="st3")
                        for nt in range(4):
                            ps = ps_p.tile([128, 512], f32, name="pp3")
                            for h in range(HEADS):
                                nc.tensor.matmul(
                                    ps[:],
                                    ots_c[h // 2][:, h % 2,
                                                  tt * 128:(tt + 1) * 128],
                                    wo[:, h, nt * 512:(nt + 1) * 512],
                                    start=(h == 0), stop=(h == HEADS - 1))
                            nc.vector.tensor_copy(
                                stg[:, nt * 512:(nt + 1) * 512], ps[:])
                        r0 = c * WIN + tt * 128
                        if c == NC_ - 1 and tt == 1:
                            # split the final store so the tail drains sooner
                            nc.sync.dma_start(OUT[r0:r0 + 128, 0:1024],
                                              stg[:, 0:1024])
                            nc.sync.dma_start(OUT[r0:r0 + 128, 1024:2048],
                                              stg[:, 1024:2048])
                        else:
                            nc.sync.dma_start(OUT[r0:r0 + 128, :], stg[:])
                    if c + 1 < NC_:
                        kt_prev, v_prev = kt_cur, v_cur
                        kt_cur, v_cur, qt = kt_next, v_next, qt_next
    return nc


def _host_inputs(hidden_states, Wq, Wk, Wv, Wo, T):
    """Build the 8 per-core input maps."""
    TH = T + WIN
    inv_freq = 1.0 / (THETA ** (np.arange(0, HD, 2, dtype=np.float32) / HD))

    qq = np.arange(WIN)[None, :]
    kk = np.arange(128)[:, None]
    # [mask for current-chunk keys 0:128 vs all 256 queries |
    #  mask for keys 128:256 vs the live queries 128:256]
    tri23 = np.concatenate(
        [(qq >= kk), (qq[:, 128:] >= kk + 128)], 1).astype(ml_dtypes.bfloat16)
    onesm_bf = np.ones((128, 128), ml_dtypes.bfloat16)

    Wq, Wk, Wv, Wo = (np.asarray(w, np.float32).astype(ml_dtypes.bfloat16)
                      for w in (Wq, Wk, Wv, Wo))
    # [in, out] -> [ob, p, ib, o] tiles (p = row within 128-input block)
    Wq, Wk = (np.ascontiguousarray(
        w.reshape(IB, 128, HEADS, 128).transpose(2, 1, 0, 3))
        for w in (Wq, Wk))
    in_maps = []
    for core in range(8):
        b, sh = divmod(core, NSH)
        t0 = sh * T
        hs = np.zeros((TH, DIMS), np.float32)
        lo = max(0, t0 - WIN)
        hs[WIN - (t0 - lo):] = hidden_states[b, lo:t0 + T]
        hT = np.ascontiguousarray(hs.T).astype(ml_dtypes.bfloat16)

        pos = np.arange(t0 - WIN, t0 + T, dtype=np.float32)
        f = np.outer(inv_freq, pos)                      # [64, TH]
        cos = np.concatenate([np.cos(f), np.cos(f)], 0)  # [128, TH]
        sin = np.sin(f)
        sins = np.concatenate([-sin, sin], 0)
        pg = np.full((128, 1), -1e30 if sh == 0 else 0.0, np.float32)
        in_maps.append({
            "HT": hT, "WQ": Wq, "WK": Wk, "WV": Wv, "WO": Wo,
            "COS": cos.astype(ml_dtypes.bfloat16),
            "SINS": sins.astype(ml_dtypes.bfloat16),
            "TRI23": tri23, "PGATE": pg, "ONESM": onesm_bf,
        })
    return in_maps


_CACHE = {}


def run(hidden_states, Wq, Wk, Wv, Wo, T=S // NSH, **spmd_kwargs):
    key = T
    if key not in _CACHE:
        nc = bacc.Bacc(None)
        build(nc, T)
        nc.finalize()
        _CACHE[key] = nc
    nc = _CACHE[key]
    in_maps = _host_inputs(hidden_states, Wq, Wk, Wv, Wo, T)
    res = run_bass_kernel_spmd(nc, in_maps, core_ids=list(range(8)), **spmd_kwargs)
    outs = [res.results[i]["OUT"] for i in range(8)]
    full = np.empty((B, NSH * T, DIMS), np.float32)
    for core in range(8):
        b, sh = divmod(core, NSH)
        full[b, sh * T:(sh + 1) * T] = np.asarray(outs[core]).astype(np.float32)
    return full, res


def kernel(hidden_states, Wq, Wk, Wv, Wo):
    out, _ = run(np.asarray(hidden_states), Wq, Wk, Wv, Wo)
    return out


# revision 50
# speedup vs baseline: 1.2926x; 1.0075x over previous
"""Block sliding-window attention on 8 TRN2 NeuronCores.

Sharding: sequence-parallel. 8 shards = (batch b in {0,1}) x (quarter s in
0..3); each core owns 2048 consecutive tokens of one batch plus a 256-token
K/V halo from the previous quarter (zeros + -inf gate for the first quarter).
No collectives: each core computes its tokens' full output rows.

Per-core pipeline (all matmuls bf16: full PE rate):
  P1: K^T/Q^T = W^T @ hiddenT (head-transposed layout, raw), V = hidden @ Wv
      (natural layout), staged through DRAM scratch. hiddenT is streamed in
      5 token-group DMAs so the first matmuls start ~7us in; per head-column
      the 16 weight tiles arrive as one DMA and stay resident.
  P2+P3 fused per 256-token chunk: RoPE on Q/K (rot-half via partition-offset
      DMA reload + pre-signed sin, cos/sin broadcast via stride-0 APs), then
      per head: S^T = K Q^T per 128-key block into one 2-bank PSUM tile,
      single exp on ACT (scale=1/sqrt(128), -1e30 bias gates the no-previous
      case), 0/1 triangular mask multiply on DVE, denominator via DVE
      pre-add + one all-ones matmul (broadcasts across partitions),
      O^T = V^T P^T, normalize with DVE reciprocal; afterwards the chunk's
      256 output rows are projected against SBUF-resident Wo and stored.
      P2 loads ride the Pool queue; stores ride the sync queue.
"""
import sys

try:
    import concourse  # noqa: F401
except ImportError:
    sys.path.insert(0, '/opt/trn_rl_repo')

import ml_dtypes
import numpy as np

import concourse.bacc as bacc
import concourse.mybir as mybir
import concourse.tile as tile
from concourse.bass_utils import run_bass_kernel_spmd

f32 = mybir.dt.float32
AF = mybir.ActivationFunctionType
bf16 = mybir.dt.bfloat16

DIMS = 2048
HEADS = 16
HD = 128           # head dim
WIN = 256          # window / chunk
B, S = 2, 8192
NSH = 4            # seq shards per batch
THETA = 10000.0
ISQ = float(1.0 / np.sqrt(HD))
IB = DIMS // 128   # 16 input-dim blocks


def tok_tiles(n):
    out, a = [], 0
    while a < n:
        w = min(512, n - a)
        out.append((a, w))
        a += w
    return out


def build(nc, T):
    """Emit the per-core program. T = local tokens (multiple of 512)."""
    TH = T + WIN                      # with halo
    NC_ = T // WIN                    # chunks
    HT = nc.dram_tensor("HT", [DIMS, TH], bf16, kind="ExternalInput")
    # WQ/WK pre-tiled on host to [ob, p, ib, o] so each head-column's
    # 16 weight tiles arrive as one fully-contiguous DMA
    WQ = nc.dram_tensor("WQ", [HEADS, 128, IB, 128], bf16,
                        kind="ExternalInput")
    WK = nc.dram_tensor("WK", [HEADS, 128, IB, 128], bf16,
                        kind="ExternalInput")
    WV = nc.dram_tensor("WV", [DIMS, DIMS], bf16, kind="ExternalInput")
    WO = nc.dram_tensor("WO", [DIMS, DIMS], bf16, kind="ExternalInput")
    COS = nc.dram_tensor("COS", [HD, TH], bf16, kind="ExternalInput")
    SINS = nc.dram_tensor("SINS", [HD, TH], bf16, kind="ExternalInput")
    TRI23 = nc.dram_tensor("TRI23", [128, WIN + 128], bf16,
                           kind="ExternalInput")
    PGATE = nc.dram_tensor("PGATE", [128, 1], f32, kind="ExternalInput")
    ONESM = nc.dram_tensor("ONESM", [128, 128], bf16, kind="ExternalInput")
    OUT = nc.dram_tensor("OUT", [T, DIMS], bf16, kind="ExternalOutput")

    QTS = nc.dram_tensor("QTS", [HEADS, HD, T], bf16)    # raw (pre-RoPE) Q^T
    KTS = nc.dram_tensor("KTS", [HEADS, HD, TH], bf16)   # raw K^T (with halo)
    VS = nc.dram_tensor("VS", [TH, DIMS], bf16)          # V natural

    with tile.TileContext(nc) as tc:
        with tc.tile_pool(name="cst", bufs=1) as cst, \
             tc.tile_pool(name="qk", bufs=2) as qk:
            tri23 = cst.tile([128, WIN + 128], bf16)
            pgate = cst.tile([128, 1], f32)
            onesm = cst.tile([128, 128], bf16)
            cosb = cst.tile([128, 1, TH], bf16)
            sinb = cst.tile([128, 1, TH], bf16)
            nc.gpsimd.dma_start(tri23[:], TRI23[:])
            nc.gpsimd.dma_start(pgate[:], PGATE[:])
            nc.gpsimd.dma_start(onesm[:], ONESM[:])
            nc.gpsimd.dma_start(cosb[:, 0], COS[:])
            nc.gpsimd.dma_start(sinb[:, 0], SINS[:])

            # ---------------- P1: projections ----------------
            with tc.tile_pool(name="p1", bufs=1) as p1, \
                 tc.tile_pool(name="wp", bufs=2) as wp, \
                 tc.tile_pool(name="st", bufs=2) as st, \
                 tc.tile_pool(name="pp", bufs=4, space="PSUM") as pp:
                def load_wt(W_, ob):
                    wt = wp.tile([128, IB, 128], bf16, name="w")
                    nc.sync.dma_start(wt[:], W_[ob])
                    return wt

                # first weight tile ahead of the big hidden-state load so
                # the PE can start as soon as token group 0 lands
                wt_next = load_wt(WK, 0)
                ht = p1.tile([128, IB, TH], bf16)
                for a, w in [(0, 256), (256, 256)] + tok_tiles(TH)[1:]:
                    nc.sync.dma_start(
                        ht[:, :, a:a + w],
                        HT[:, a:a + w].rearrange("(ib p) t -> p ib t", p=128))

                # K^T / Q^T: lhsT = W column block [128in, 128out].
                # RoPE is applied here, once per head row: rotate-half via
                # SBUF->SBUF partition-swap DMA, then 3 in-place DVE ops
                # (DVE is otherwise idle in P1). pos0 = column into COS/SINS.
                seq = [(WK, KTS, 0, TH, ob) for ob in range(HEADS)] + \
                      [(WQ, QTS, WIN, T, ob) for ob in range(HEADS)]
                for idx, (W_, DST, t0, tlen, ob) in enumerate(seq):
                    wt = wt_next
                    if idx + 1 < len(seq):
                        nw, _, _, _, nob = seq[idx + 1]
                        wt_next = load_wt(nw, nob)
                    stg = st.tile([128, TH], bf16, name="stg")
                    for a, w in tok_tiles(tlen):
                        ps = pp.tile([128, 512], f32, name="pp")
                        for ib in range(IB):
                            nc.tensor.matmul(
                                ps[:, :w], wt[:, ib],
                                ht[:, ib, t0 + a:t0 + a + w],
                                start=(ib == 0), stop=(ib == IB - 1))
                        nc.scalar.copy(stg[:, a:a + w], ps[:, :w])
                    rot = st.tile([128, TH], bf16, name="rot", bufs=1)
                    nc.sync.dma_start(rot[0:64, 0:tlen], stg[64:128, 0:tlen])
                    nc.sync.dma_start(rot[64:128, 0:tlen], stg[0:64, 0:tlen])
                    cs = cosb[:, 0, t0:t0 + tlen]
                    sn = sinb[:, 0, t0:t0 + tlen]
                    nc.vector.tensor_mul(rot[:, 0:tlen], rot[:, 0:tlen], sn)
                    nc.vector.tensor_mul(stg[:, 0:tlen], stg[:, 0:tlen], cs)
                    nc.vector.tensor_add(stg[:, 0:tlen], stg[:, 0:tlen],
                                         rot[:, 0:tlen])
                    nc.sync.dma_start(DST[ob][:, 0:tlen], stg[:, 0:tlen])

                # V natural: lhsT = hT block [128in, 128tok], rhs = Wv rows
                def load_wv(og):
                    wv = wp.tile([128, IB, 512], bf16, name="wv")
                    nc.sync.dma_start(
                        wv[:], WV[:, og * 512:(og + 1) * 512]
                        .rearrange("(ib p) d -> p ib d", p=128))
                    return wv

                wv_next = load_wv(0)
                for og in range(4):
                    wv = wv_next
                    if og + 1 < 4:
                        wv_next = load_wv(og + 1)
                    for tb in range(TH // 128):
                        ps = pp.tile([128, 512], f32, name="pp")
                        for ib in range(IB):
                            nc.tensor.matmul(
                                ps[:], ht[:, ib, tb * 128:(tb + 1) * 128],
                                wv[:, ib, :],
                                start=(ib == 0), stop=(ib == IB - 1))
                        stgv = st.tile([128, 512], bf16, name="stgv")
                        nc.scalar.copy(stgv[:], ps[:])
                        nc.sync.dma_start(
                            VS[tb * 128:(tb + 1) * 128,
                               og * 512:(og + 1) * 512], stgv[:])

            # ---------------- P2 + P3 fused ----------------
            with tc.tile_pool(name="wop", bufs=1) as wop, \
                 tc.tile_pool(name="pbp", bufs=4) as pbp, \
                 tc.tile_pool(name="pad", bufs=2) as padp, \
                 tc.tile_pool(name="pad2", bufs=4) as padp2, \
                 tc.tile_pool(name="ob", bufs=2) as obp, \
                 tc.tile_pool(name="otp", bufs=18) as otp, \
                 tc.tile_pool(name="st3", bufs=2) as st3, \
                 tc.tile_pool(name="ps_s", bufs=2, space="PSUM") as ps_s, \
                 tc.tile_pool(name="ps_po", bufs=1, space="PSUM") as ps_po, \
                 tc.tile_pool(name="ps_p", bufs=2, space="PSUM") as ps_p:

                def kq_issue(SRC, c0, which):
                    t = qk.tile([128, HEADS, WIN], bf16, name=which,
                                bufs=3 if which == "kt" else 2)
                    nc.gpsimd.dma_start(
                        t[:], SRC[:, :, c0:c0 + WIN]
                        .rearrange("h d w -> d h w"))
                    return t

                def v_issue(w0):
                    v = qk.tile([128, 2, DIMS], bf16, name="v", bufs=3)
                    nc.gpsimd.dma_start(
                        v[:], VS[w0:w0 + WIN].rearrange("(tb p) c -> p tb c",
                                                        p=128))
                    return v

                # prologue: halo window + chunk-0 tiles. K/Q loads first
                # (their scratch is ready mid-P1, so these drain early); the
                # V loads and the big Wo load go behind them in the queue.
                kt_prev = kq_issue(KTS, 0, "kt")
                kt_cur = kq_issue(KTS, WIN, "kt")
                qt = kq_issue(QTS, 0, "qt")
                v_prev = v_issue(0)
                v_cur = v_issue(WIN)
                wo = wop.tile([128, IB, DIMS], bf16)
                for nt in range(4):
                    nc.gpsimd.dma_start(
                        wo[:, :, nt * 512:(nt + 1) * 512],
                        WO[:, nt * 512:(nt + 1) * 512]
                        .rearrange("(ib p) d -> p ib d", p=128))

                def p3_emit(c, ots_c):
                    # P3: project chunk c's 256 output rows against Wo.
                    # Called one chunk late so the accumulation never waits
                    # on the freshly-written normalize chain.
                    for tt in range(2):
                        stg = st3.tile([128, DIMS], bf16, name="st3")
                        for nt in range(4):
                            ps = ps_p.tile([128, 512], f32, name="pp3")
                            for h in range(HEADS):
                                nc.tensor.matmul(
                                    ps[:],
                                    ots_c[h // 2][:, h % 2,
                                                  tt * 128:(tt + 1) * 128],
                                    wo[:, h, nt * 512:(nt + 1) * 512],
                                    start=(h == 0), stop=(h == HEADS - 1))
                            nc.vector.tensor_copy(
                                stg[:, nt * 512:(nt + 1) * 512], ps[:])
                        r0 = c * WIN + tt * 128
                        if c == NC_ - 1 and tt == 1:
                            # split the final store so the tail drains sooner
                            nc.sync.dma_start(OUT[r0:r0 + 128, 0:1024],
                                              stg[:, 0:1024])
                            nc.sync.dma_start(OUT[r0:r0 + 128, 1024:2048],
                                              stg[:, 1024:2048])
                        else:
                            nc.sync.dma_start(OUT[r0:r0 + 128, :], stg[:])

                W2 = 2 * WIN
                p3_prev = None
                for c in range(NC_):
                    if c + 1 < NC_:
                        kt_next = kq_issue(KTS, WIN + (c + 1) * WIN, "kt")
                        qt_next = kq_issue(QTS, (c + 1) * WIN, "qt")
                        v_next = v_issue(WIN + (c + 1) * WIN)

                    kts = [kt_prev, kt_prev, kt_cur, kt_cur]
                    vs = [v_prev, v_prev, v_cur, v_cur]
                    ots_c = []
                    pend = None  # (h0, [(h, pb, pa2) x2]) awaiting den+O

                    def den_o(pair):
                        pod = ps_po.tile([128, 4, WIN], f32, name="pod")
                        for i, (h, pb, pa2) in enumerate(pair):
                            nc.tensor.matmul(pod[:, 2 + i], onesm[:], pa2[:],
                                             start=True, stop=True)
                            for kb in range(3):
                                nc.tensor.matmul(
                                    pod[:, i],
                                    vs[kb][:, kb % 2, h * 128:(h + 1) * 128],
                                    pb[:, kb * WIN:(kb + 1) * WIN],
                                    start=(kb == 0), stop=(kb == 2))
                            nc.tensor.matmul(
                                pod[:, i, 128:WIN],
                                vs[3][:, 1, h * 128:(h + 1) * 128],
                                pb[:, 3 * WIN:3 * WIN + 128], start=False,
                                stop=True, skip_group_check=True)
                        rb = obp.tile([128, 2, WIN], f32, name="rb")
                        nc.vector.reciprocal_approx_fast(rb[:], pod[:, 2:4])
                        ot = otp.tile([128, 2, WIN], bf16, name="ot")
                        nc.vector.tensor_mul(ot[:], pod[:, 0:2], rb[:])
                        ots_c.append(ot)

                    for h0 in range(0, HEADS, 2):
                        pair = []
                        for h in (h0, h0 + 1):
                            # scores packed flat in one 2-bank tile; key
                            # block 3 only sees queries 128:256 (queries
                            # 0:128 are fully causal-masked against keys
                            # 128:256), packed at cols 768:896 so a single
                            # exp and a single mask op cover everything
                            ps = ps_s.tile([128, 4 * WIN], f32, name="ps")
                            for kb in range(3):
                                nc.tensor.matmul(
                                    ps[:, kb * WIN:(kb + 1) * WIN],
                                    kts[kb][:, h,
                                            (kb % 2) * 128:(kb % 2) * 128 + 128],
                                    qt[:, h], start=True, stop=True)
                            nc.tensor.matmul(
                                ps[:, 3 * WIN:3 * WIN + 128],
                                kts[3][:, h, 128:256],
                                qt[:, h, 128:WIN], start=True, stop=True)
                            pb = pbp.tile([128, 4 * WIN], bf16, name="pb")
                            if c == 0:
                                nc.scalar.activation(
                                    pb[:, 0:2 * WIN], ps[:, 0:2 * WIN],
                                    AF.Exp, bias=pgate[:], scale=ISQ)
                                nc.scalar.activation(
                                    pb[:, 2 * WIN:3 * WIN + 128],
                                    ps[:, 2 * WIN:3 * WIN + 128],
                                    AF.Exp, scale=ISQ)
                            else:
                                nc.scalar.activation(
                                    pb[:, 0:3 * WIN + 128],
                                    ps[:, 0:3 * WIN + 128], AF.Exp, scale=ISQ)
                            nc.vector.tensor_mul(
                                pb[:, 2 * WIN:3 * WIN + 128],
                                pb[:, 2 * WIN:3 * WIN + 128], tri23[:])
                            # denominator pre-add on DVE (kb3 live half only)
                            pa = padp.tile([128, WIN], bf16, name="pa")
                            nc.vector.tensor_add(pa[:], pb[:, 0:WIN],
                                                 pb[:, WIN:2 * WIN])
                            pa2 = padp2.tile([128, WIN], bf16, name="pa2")
                            nc.vector.tensor_add(pa2[:], pa[:],
                                                 pb[:, 2 * WIN:3 * WIN])
                            nc.vector.tensor_add(
                                pa2[:, 128:WIN], pa2[:, 128:WIN],
                                pb[:, 3 * WIN:3 * WIN + 128])
                            pair.append((h, pb, pa2))
                        if pend is not None:
                            den_o(pend)
                        pend = pair
                    den_o(pend)

                    if p3_prev is not None:
                        p3_emit(c - 1, p3_prev)
                    p3_prev = ots_c
                    if c + 1 < NC_:
                        kt_prev, v_prev = kt_cur, v_cur
                        kt_cur, v_cur, qt = kt_next, v_next, qt_next
                p3_emit(NC_ - 1, p3_prev)
    return nc


def _host_inputs(hidden_states, Wq, Wk, Wv, Wo, T):
    """Build the 8 per-core input maps."""
    TH = T + WIN
    inv_freq = 1.0 / (THETA ** (np.arange(0, HD, 2, dtype=np.float32) / HD))

    qq = np.arange(WIN)[None, :]
    kk = np.arange(128)[:, None]
    # [mask for current-chunk keys 0:128 vs all 256 queries |
    #  mask for keys 128:256 vs the live queries 128:256]
    tri23 = np.concatenate(
        [(qq >= kk), (qq[:, 128:] >= kk + 128)], 1).astype(ml_dtypes.bfloat16)
    onesm_bf = np.ones((128, 128), ml_dtypes.bfloat16)

    Wq, Wk, Wv, Wo = (np.asarray(w, np.float32).astype(ml_dtypes.bfloat16)
                      for w in (Wq, Wk, Wv, Wo))
    # [in, out] -> [ob, p, ib, o] tiles (p = row within 128-input block)
    Wq, Wk = (np.ascontiguousarray(
        w.reshape(IB, 128, HEADS, 128).transpose(2, 1, 0, 3))
        for w in (Wq, Wk))
    in_maps = []
    for core in range(8):
        b, sh = divmod(core, NSH)
        t0 = sh * T
        hs = np.zeros((TH, DIMS), np.float32)
        lo = max(0, t0 - WIN)
        hs[WIN - (t0 - lo):] = hidden_states[b, lo:t0 + T]
        hT = np.ascontiguousarray(hs.T).astype(ml_dtypes.bfloat16)

        pos = np.arange(t0 - WIN, t0 + T, dtype=np.float32)
        f = np.outer(inv_freq, pos)                      # [64, TH]
        cos = np.concatenate([np.cos(f), np.cos(f)], 0)  # [128, TH]
        sin = np.sin(f)
        sins = np.concatenate([-sin, sin], 0)
        pg = np.full((128, 1), -1e30 if sh == 0 else 0.0, np.float32)
        in_maps.append({
            "HT": hT, "WQ": Wq, "WK": Wk, "WV": Wv, "WO": Wo,
            "COS": cos.astype(ml_dtypes.bfloat16),
            "SINS": sins.astype(ml_dtypes.bfloat16),
            "TRI23": tri23, "PGATE": pg, "ONESM": onesm_bf,
        })
    return in_maps


_CACHE = {}


def run(hidden_states, Wq, Wk, Wv, Wo, T=S // NSH, **spmd_kwargs):
    key = T
    if key not in _CACHE:
        nc = bacc.Bacc(None)
        build(nc, T)
        nc.finalize()
        _CACHE[key] = nc
    nc = _CACHE[key]
    in_maps = _host_inputs(hidden_states, Wq, Wk, Wv, Wo, T)
    res = run_bass_kernel_spmd(nc, in_maps, core_ids=list(range(8)), **spmd_kwargs)
    outs = [res.results[i]["OUT"] for i in range(8)]
    full = np.empty((B, NSH * T, DIMS), np.float32)
    for core in range(8):
        b, sh = divmod(core, NSH)
        full[b, sh * T:(sh + 1) * T] = np.asarray(outs[core]).astype(np.float32)
    return full, res


def kernel(hidden_states, Wq, Wk, Wv, Wo):
    out, _ = run(np.asarray(hidden_states), Wq, Wk, Wv, Wo)
    return out


# revision 53
# speedup vs baseline: 1.2953x; 1.0021x over previous
"""Block sliding-window attention on 8 TRN2 NeuronCores.

Sharding: sequence-parallel. 8 shards = (batch b in {0,1}) x (quarter s in
0..3); each core owns 2048 consecutive tokens of one batch plus a 256-token
K/V halo from the previous quarter (zeros + -inf gate for the first quarter).
No collectives: each core computes its tokens' full output rows.

Per-core pipeline (all matmuls bf16: full PE rate):
  P1: K^T/Q^T = W^T @ hiddenT (head-transposed layout, raw), V = hidden @ Wv
      (natural layout), staged through DRAM scratch. hiddenT is streamed in
      5 token-group DMAs so the first matmuls start ~7us in; per head-column
      the 16 weight tiles arrive as one DMA and stay resident.
  P2+P3 fused per 256-token chunk: RoPE on Q/K (rot-half via partition-offset
      DMA reload + pre-signed sin, cos/sin broadcast via stride-0 APs), then
      per head: S^T = K Q^T per 128-key block into one 2-bank PSUM tile,
      single exp on ACT (scale=1/sqrt(128), -1e30 bias gates the no-previous
      case), 0/1 triangular mask multiply on DVE, denominator via DVE
      pre-add + one all-ones matmul (broadcasts across partitions),
      O^T = V^T P^T, normalize with DVE reciprocal; afterwards the chunk's
      256 output rows are projected against SBUF-resident Wo and stored.
      P2 loads ride the Pool queue; stores ride the sync queue.
"""
import sys

try:
    import concourse  # noqa: F401
except ImportError:
    sys.path.insert(0, '/opt/trn_rl_repo')

import ml_dtypes
import numpy as np

import concourse.bacc as bacc
import concourse.mybir as mybir
import concourse.tile as tile
from concourse.bass_utils import run_bass_kernel_spmd

f32 = mybir.dt.float32
AF = mybir.ActivationFunctionType
bf16 = mybir.dt.bfloat16

DIMS = 2048
HEADS = 16
HD = 128           # head dim
WIN = 256          # window / chunk
B, S = 2, 8192
NSH = 4            # seq shards per batch
THETA = 10000.0
ISQ = float(1.0 / np.sqrt(HD))
IB = DIMS // 128   # 16 input-dim blocks


def tok_tiles(n):
    out, a = [], 0
    while a < n:
        w = min(512, n - a)
        out.append((a, w))
        a += w
    return out


def build(nc, T):
    """Emit the per-core program. T = local tokens (multiple of 512)."""
    TH = T + WIN                      # with halo
    NC_ = T // WIN                    # chunks
    HT = nc.dram_tensor("HT", [DIMS, TH], bf16, kind="ExternalInput")
    # WQ/WK pre-tiled on host to [ob, p, ib, o] so each head-column's
    # 16 weight tiles arrive as one fully-contiguous DMA
    WQ = nc.dram_tensor("WQ", [HEADS, 128, IB, 128], bf16,
                        kind="ExternalInput")
    WK = nc.dram_tensor("WK", [HEADS, 128, IB, 128], bf16,
                        kind="ExternalInput")
    WV = nc.dram_tensor("WV", [DIMS, DIMS], bf16, kind="ExternalInput")
    WO = nc.dram_tensor("WO", [DIMS, DIMS], bf16, kind="ExternalInput")
    COS = nc.dram_tensor("COS", [HD, TH], bf16, kind="ExternalInput")
    SINS = nc.dram_tensor("SINS", [HD, TH], bf16, kind="ExternalInput")
    TRI23 = nc.dram_tensor("TRI23", [128, WIN + 128], bf16,
                           kind="ExternalInput")
    PGATE = nc.dram_tensor("PGATE", [128, 1], f32, kind="ExternalInput")
    ONESM = nc.dram_tensor("ONESM", [128, 128], bf16, kind="ExternalInput")
    OUT = nc.dram_tensor("OUT", [T, DIMS], bf16, kind="ExternalOutput")

    QTS = nc.dram_tensor("QTS", [HEADS, HD, T], bf16)    # raw (pre-RoPE) Q^T
    KTS = nc.dram_tensor("KTS", [HEADS, HD, TH], bf16)   # raw K^T (with halo)
    VS = nc.dram_tensor("VS", [TH, DIMS], bf16)          # V natural

    with tile.TileContext(nc) as tc:
        with tc.tile_pool(name="cst", bufs=1) as cst, \
             tc.tile_pool(name="qk", bufs=2) as qk:
            tri23 = cst.tile([128, WIN + 128], bf16)
            pgate = cst.tile([128, 1], f32)
            onesm = cst.tile([128, 128], bf16)
            cosb = cst.tile([128, 1, TH], bf16)
            sinb = cst.tile([128, 1, TH], bf16)
            nc.gpsimd.dma_start(onesm[:], ONESM[:])
            nc.gpsimd.dma_start(tri23[:], TRI23[:])
            nc.gpsimd.dma_start(pgate[:], PGATE[:])
            nc.gpsimd.dma_start(cosb[:, 0], COS[:])
            nc.gpsimd.dma_start(sinb[:, 0], SINS[:])

            # PE warmup while the first hidden-state groups are in flight:
            # keeps the PE continuously busy so the real matmuls start at
            # full clock instead of ramping from the low p-state
            with tc.tile_pool(name="wu", bufs=1, space="PSUM") as wu:
                wps = wu.tile([128, 128], f32)
                NWU = 48
                for i in range(NWU):
                    nc.tensor.matmul(wps[:], onesm[:], onesm[:],
                                     start=(i == 0), stop=(i == NWU - 1))

            # ---------------- P1: projections ----------------
            with tc.tile_pool(name="p1", bufs=1) as p1, \
                 tc.tile_pool(name="wp", bufs=2) as wp, \
                 tc.tile_pool(name="st", bufs=2) as st, \
                 tc.tile_pool(name="pp", bufs=4, space="PSUM") as pp:
                def load_wt(W_, ob):
                    wt = wp.tile([128, IB, 128], bf16, name="w")
                    nc.sync.dma_start(wt[:], W_[ob])
                    return wt

                # first weight tile ahead of the big hidden-state load so
                # the PE can start as soon as token group 0 lands
                wt_next = load_wt(WK, 0)
                ht = p1.tile([128, IB, TH], bf16)
                for a, w in [(0, 256), (256, 256)] + tok_tiles(TH)[1:]:
                    nc.sync.dma_start(
                        ht[:, :, a:a + w],
                        HT[:, a:a + w].rearrange("(ib p) t -> p ib t", p=128))

                # K^T / Q^T: lhsT = W column block [128in, 128out].
                # RoPE is applied here, once per head row: rotate-half via
                # SBUF->SBUF partition-swap DMA, then 3 in-place DVE ops
                # (DVE is otherwise idle in P1). pos0 = column into COS/SINS.
                seq = [(WK, KTS, 0, TH, ob) for ob in range(HEADS)] + \
                      [(WQ, QTS, WIN, T, ob) for ob in range(HEADS)]
                for idx, (W_, DST, t0, tlen, ob) in enumerate(seq):
                    wt = wt_next
                    if idx + 1 < len(seq):
                        nw, _, _, _, nob = seq[idx + 1]
                        wt_next = load_wt(nw, nob)
                    stg = st.tile([128, TH], bf16, name="stg")
                    for a, w in tok_tiles(tlen):
                        ps = pp.tile([128, 512], f32, name="pp")
                        for ib in range(IB):
                            nc.tensor.matmul(
                                ps[:, :w], wt[:, ib],
                                ht[:, ib, t0 + a:t0 + a + w],
                                start=(ib == 0), stop=(ib == IB - 1))
                        nc.scalar.copy(stg[:, a:a + w], ps[:, :w])
                    rot = st.tile([128, TH], bf16, name="rot", bufs=1)
                    nc.sync.dma_start(rot[0:64, 0:tlen], stg[64:128, 0:tlen])
                    nc.sync.dma_start(rot[64:128, 0:tlen], stg[0:64, 0:tlen])
                    cs = cosb[:, 0, t0:t0 + tlen]
                    sn = sinb[:, 0, t0:t0 + tlen]
                    nc.vector.tensor_mul(rot[:, 0:tlen], rot[:, 0:tlen], sn)
                    nc.vector.tensor_mul(stg[:, 0:tlen], stg[:, 0:tlen], cs)
                    nc.vector.tensor_add(stg[:, 0:tlen], stg[:, 0:tlen],
                                         rot[:, 0:tlen])
                    nc.sync.dma_start(DST[ob][:, 0:tlen], stg[:, 0:tlen])

                # V natural: lhsT = hT block [128in, 128tok], rhs = Wv rows
                def load_wv(og):
                    wv = wp.tile([128, IB, 512], bf16, name="wv")
                    nc.sync.dma_start(
                        wv[:], WV[:, og * 512:(og + 1) * 512]
                        .rearrange("(ib p) d -> p ib d", p=128))
                    return wv

                wv_next = load_wv(0)
                for og in range(4):
                    wv = wv_next
                    if og + 1 < 4:
                        wv_next = load_wv(og + 1)
                    for tb in range(TH // 128):
                        ps = pp.tile([128, 512], f32, name="pp")
                        for ib in range(IB):
                            nc.tensor.matmul(
                                ps[:], ht[:, ib, tb * 128:(tb + 1) * 128],
                                wv[:, ib, :],
                                start=(ib == 0), stop=(ib == IB - 1))
                        stgv = st.tile([128, 512], bf16, name="stgv")
                        nc.scalar.copy(stgv[:], ps[:])
                        nc.sync.dma_start(
                            VS[tb * 128:(tb + 1) * 128,
                               og * 512:(og + 1) * 512], stgv[:])

            # ---------------- P2 + P3 fused ----------------
            with tc.tile_pool(name="wop", bufs=1) as wop, \
                 tc.tile_pool(name="pbp", bufs=4) as pbp, \
                 tc.tile_pool(name="pad", bufs=2) as padp, \
                 tc.tile_pool(name="pad2", bufs=4) as padp2, \
                 tc.tile_pool(name="ob", bufs=2) as obp, \
                 tc.tile_pool(name="otp", bufs=18) as otp, \
                 tc.tile_pool(name="st3", bufs=2) as st3, \
                 tc.tile_pool(name="ps_s", bufs=2, space="PSUM") as ps_s, \
                 tc.tile_pool(name="ps_po", bufs=1, space="PSUM") as ps_po, \
                 tc.tile_pool(name="ps_p", bufs=2, space="PSUM") as ps_p:

                def kq_issue(SRC, c0, which):
                    t = qk.tile([128, HEADS, WIN], bf16, name=which,
                                bufs=3 if which == "kt" else 2)
                    nc.gpsimd.dma_start(
                        t[:], SRC[:, :, c0:c0 + WIN]
                        .rearrange("h d w -> d h w"))
                    return t

                def v_issue(w0):
                    v = qk.tile([128, 2, DIMS], bf16, name="v", bufs=3)
                    nc.gpsimd.dma_start(
                        v[:], VS[w0:w0 + WIN].rearrange("(tb p) c -> p tb c",
                                                        p=128))
                    return v

                # prologue: halo window + chunk-0 tiles. K/Q loads first
                # (their scratch is ready mid-P1, so these drain early); the
                # V loads and the big Wo load go behind them in the queue.
                kt_prev = kq_issue(KTS, 0, "kt")
                kt_cur = kq_issue(KTS, WIN, "kt")
                qt = kq_issue(QTS, 0, "qt")
                v_prev = v_issue(0)
                v_cur = v_issue(WIN)
                wo = wop.tile([128, IB, DIMS], bf16)
                for nt in range(4):
                    nc.gpsimd.dma_start(
                        wo[:, :, nt * 512:(nt + 1) * 512],
                        WO[:, nt * 512:(nt + 1) * 512]
                        .rearrange("(ib p) d -> p ib d", p=128))

                def p3_emit(c, ots_c):
                    # P3: project chunk c's 256 output rows against Wo.
                    # Called one chunk late so the accumulation never waits
                    # on the freshly-written normalize chain.
                    for tt in range(2):
                        stg = st3.tile([128, DIMS], bf16, name="st3")
                        for nt in range(4):
                            ps = ps_p.tile([128, 512], f32, name="pp3")
                            for h in range(HEADS):
                                nc.tensor.matmul(
                                    ps[:],
                                    ots_c[h // 2][:, h % 2,
                                                  tt * 128:(tt + 1) * 128],
                                    wo[:, h, nt * 512:(nt + 1) * 512],
                                    start=(h == 0), stop=(h == HEADS - 1))
                            nc.vector.tensor_copy(
                                stg[:, nt * 512:(nt + 1) * 512], ps[:])
                        r0 = c * WIN + tt * 128
                        if c == NC_ - 1 and tt == 1:
                            # split the final store so the tail drains sooner
                            nc.sync.dma_start(OUT[r0:r0 + 128, 0:1024],
                                              stg[:, 0:1024])
                            nc.sync.dma_start(OUT[r0:r0 + 128, 1024:2048],
                                              stg[:, 1024:2048])
                        else:
                            nc.sync.dma_start(OUT[r0:r0 + 128, :], stg[:])

                W2 = 2 * WIN
                p3_prev = None
                for c in range(NC_):
                    if c + 1 < NC_:
                        kt_next = kq_issue(KTS, WIN + (c + 1) * WIN, "kt")
                        qt_next = kq_issue(QTS, (c + 1) * WIN, "qt")
                        v_next = v_issue(WIN + (c + 1) * WIN)

                    kts = [kt_prev, kt_prev, kt_cur, kt_cur]
                    vs = [v_prev, v_prev, v_cur, v_cur]
                    ots_c = []
                    pend = None  # (h0, [(h, pb, pa2) x2]) awaiting den+O

                    def den_o(pair):
                        pod = ps_po.tile([128, 4, WIN], f32, name="pod")
                        for i, (h, pb, pa2) in enumerate(pair):
                            nc.tensor.matmul(pod[:, 2 + i], onesm[:], pa2[:],
                                             start=True, stop=True)
                            for kb in range(3):
                                nc.tensor.matmul(
                                    pod[:, i],
                                    vs[kb][:, kb % 2, h * 128:(h + 1) * 128],
                                    pb[:, kb * WIN:(kb + 1) * WIN],
                                    start=(kb == 0), stop=(kb == 2))
                            nc.tensor.matmul(
                                pod[:, i, 128:WIN],
                                vs[3][:, 1, h * 128:(h + 1) * 128],
                                pb[:, 3 * WIN:3 * WIN + 128], start=False,
                                stop=True, skip_group_check=True)
                        rb = obp.tile([128, 2, WIN], f32, name="rb")
                        nc.vector.reciprocal_approx_fast(rb[:], pod[:, 2:4])
                        ot = otp.tile([128, 2, WIN], bf16, name="ot")
                        nc.vector.tensor_mul(ot[:], pod[:, 0:2], rb[:])
                        ots_c.append(ot)

                    for h0 in range(0, HEADS, 2):
                        pair = []
                        for h in (h0, h0 + 1):
                            # scores packed flat in one 2-bank tile; key
                            # block 3 only sees queries 128:256 (queries
                            # 0:128 are fully causal-masked against keys
                            # 128:256), packed at cols 768:896 so a single
                            # exp and a single mask op cover everything
                            ps = ps_s.tile([128, 4 * WIN], f32, name="ps")
                            for kb in range(3):
                                nc.tensor.matmul(
                                    ps[:, kb * WIN:(kb + 1) * WIN],
                                    kts[kb][:, h,
                                            (kb % 2) * 128:(kb % 2) * 128 + 128],
                                    qt[:, h], start=True, stop=True)
                            nc.tensor.matmul(
                                ps[:, 3 * WIN:3 * WIN + 128],
                                kts[3][:, h, 128:256],
                                qt[:, h, 128:WIN], start=True, stop=True)
                            pb = pbp.tile([128, 4 * WIN], bf16, name="pb")
                            if c == 0:
                                nc.scalar.activation(
                                    pb[:, 0:2 * WIN], ps[:, 0:2 * WIN],
                                    AF.Exp, bias=pgate[:], scale=ISQ)
                                nc.scalar.activation(
                                    pb[:, 2 * WIN:3 * WIN + 128],
                                    ps[:, 2 * WIN:3 * WIN + 128],
                                    AF.Exp, scale=ISQ)
                            else:
                                nc.scalar.activation(
                                    pb[:, 0:3 * WIN + 128],
                                    ps[:, 0:3 * WIN + 128], AF.Exp, scale=ISQ)
                            nc.vector.tensor_mul(
                                pb[:, 2 * WIN:3 * WIN + 128],
                                pb[:, 2 * WIN:3 * WIN + 128], tri23[:])
                            # denominator pre-add on DVE (kb3 live half only)
                            pa = padp.tile([128, WIN], bf16, name="pa")
                            nc.vector.tensor_add(pa[:], pb[:, 0:WIN],
                                                 pb[:, WIN:2 * WIN])
                            pa2 = padp2.tile([128, WIN], bf16, name="pa2")
                            nc.vector.tensor_add(pa2[:], pa[:],
                                                 pb[:, 2 * WIN:3 * WIN])
                            nc.vector.tensor_add(
                                pa2[:, 128:WIN], pa2[:, 128:WIN],
                                pb[:, 3 * WIN:3 * WIN + 128])
                            pair.append((h, pb, pa2))
                        if pend is not None:
                            den_o(pend)
                        pend = pair
                    den_o(pend)

                    if p3_prev is not None:
                        p3_emit(c - 1, p3_prev)
                    p3_prev = ots_c
                    if c + 1 < NC_:
                        kt_prev, v_prev = kt_cur, v_cur
                        kt_cur, v_cur, qt = kt_next, v_next, qt_next
                p3_emit(NC_ - 1, p3_prev)
    return nc


def _host_inputs(hidden_states, Wq, Wk, Wv, Wo, T):
    """Build the 8 per-core input maps."""
    TH = T + WIN
    inv_freq = 1.0 / (THETA ** (np.arange(0, HD, 2, dtype=np.float32) / HD))

    qq = np.arange(WIN)[None, :]
    kk = np.arange(128)[:, None]
    # [mask for current-chunk keys 0:128 vs all 256 queries |
    #  mask for keys 128:256 vs the live queries 128:256]
    tri23 = np.concatenate(
        [(qq >= kk), (qq[:, 128:] >= kk + 128)], 1).astype(ml_dtypes.bfloat16)
    onesm_bf = np.ones((128, 128), ml_dtypes.bfloat16)

    Wq, Wk, Wv, Wo = (np.asarray(w, np.float32).astype(ml_dtypes.bfloat16)
                      for w in (Wq, Wk, Wv, Wo))
    # [in, out] -> [ob, p, ib, o] tiles (p = row within 128-input block)
    Wq, Wk = (np.ascontiguousarray(
        w.reshape(IB, 128, HEADS, 128).transpose(2, 1, 0, 3))
        for w in (Wq, Wk))
    in_maps = []
    for core in range(8):
        b, sh = divmod(core, NSH)
        t0 = sh * T
        hs = np.zeros((TH, DIMS), np.float32)
        lo = max(0, t0 - WIN)
        hs[WIN - (t0 - lo):] = hidden_states[b, lo:t0 + T]
        hT = np.ascontiguousarray(hs.T).astype(ml_dtypes.bfloat16)

        pos = np.arange(t0 - WIN, t0 + T, dtype=np.float32)
        f = np.outer(inv_freq, pos)                      # [64, TH]
        cos = np.concatenate([np.cos(f), np.cos(f)], 0)  # [128, TH]
        sin = np.sin(f)
        sins = np.concatenate([-sin, sin], 0)
        pg = np.full((128, 1), -1e30 if sh == 0 else 0.0, np.float32)
        in_maps.append({
            "HT": hT, "WQ": Wq, "WK": Wk, "WV": Wv, "WO": Wo,
            "COS": cos.astype(ml_dtypes.bfloat16),
            "SINS": sins.astype(ml_dtypes.bfloat16),
            "TRI23": tri23, "PGATE": pg, "ONESM": onesm_bf,
        })
    return in_maps


_CACHE = {}


def run(hidden_states, Wq, Wk, Wv, Wo, T=S // NSH, **spmd_kwargs):
    key = T
    if key not in _CACHE:
        nc = bacc.Bacc(None)
        build(nc, T)
        nc.finalize()
        _CACHE[key] = nc
    nc = _CACHE[key]
    in_maps = _host_inputs(hidden_states, Wq, Wk, Wv, Wo, T)
    res = run_bass_kernel_spmd(nc, in_maps, core_ids=list(range(8)), **spmd_kwargs)
    outs = [res.results[i]["OUT"] for i in range(8)]
    full = np.empty((B, NSH * T, DIMS), np.float32)
    for core in range(8):
        b, sh = divmod(core, NSH)
        full[b, sh * T:(sh + 1) * T] = np.asarray(outs[core]).astype(np.float32)
    return full, res


def kernel(hidden_states, Wq, Wk, Wv, Wo):
    out, _ = run(np.asarray(hidden_states), Wq, Wk, Wv, Wo)
    return out
